# revision 8
# baseline (speedup 1.0000x reference)
"""AdaGNN (gnn_message_passing) distributed Bass kernel for 8 TRN2 NeuronCores.

Math refactoring (exact, up to fp reassociation):
  The reference runs 3 PolyConvs, each applying the unnormalized Laplacian
  twice (6 gather+segment_sum rounds).  All 3 convs start from the same h and
  the per-feature diagonal scales (ld) commute through the adjacency A, so
  only TWO aggregations are needed:
     M1 = A @ (h * d)          (d = deg^-1/2 per node)
     M2 = A @ (M1 * d^2)
  With B1 = M1*d, B2 = M2*d every conv output is
     h_i = th0*(h*ld_i0)@cW_i^T + cb_i + c_h(i)*h + B1*alpha_i + B2*beta_i
  and emb = relu(concat_i(h_i) @ W3^T + b3) collapses to
     emb = relu(h @ K01 + B1 @ KB1 + B2 @ KB2 + b_emb)
  with K01/KB1/KB2/b_emb folded on the host from the (tiny) parameters.

Distribution: nodes are degree-sorted and snake-assigned to the 8 cores
(dst ownership).  Each core computes its h/x shard, AllGathers the x table
([8*(NLOC+1), 64] f32, one zero row per shard), then gathers per-edge rows
with dma_gather (4 int16 windows, per-window lane-balanced slot grids, pads
point at the window's zero row) and segment-sums with dma_scatter_add's
SBUF-parity CCE accumulate (same dst always in the same lane -> same DMA
engine -> no RMW race; consecutive scatter chunks are serialized by Tile).
"""

import numpy as np
import ml_dtypes

import concourse.bass as bass
import concourse.mybir as mybir
import concourse.tile as tile
import concourse.bacc as bacc
from concourse.bass_utils import run_bass_kernel_spmd
from concourse.masks import make_identity

F32 = mybir.dt.float32
BF16 = mybir.dt.bfloat16
I16 = mybir.dt.int16
BF16NP = ml_dtypes.bfloat16

NCORES = 8
H = 64
C_OUT = 2
THETAS = ((3.0, -3.0, 0.75), (0.0, 3.0, -1.5), (0.0, 0.0, 0.75))
CHUNK_COLS = 63                      # slot columns per chunk (<=1009 ring descs)
CHUNK_SLOTS = CHUNK_COLS * 128
IDXW = CHUNK_SLOTS // 16             # free dim of a wrapped idx tile
MINSP = 16                           # min same-dst column spacing (CCE RMW hazard)


# ---------------------------------------------------------------- host prep

def _snake(n_items, n_bins):
    """rank -> bin, serpentine."""
    r = np.arange(n_items)
    blk, pos = r // n_bins, r % n_bins
    return np.where(blk % 2 == 0, pos, n_bins - 1 - pos)


def _schedule_lane(nodes, cnts):
    """Order a lane's messages so same-node emissions are >= MINSP cols
    apart.  Returns a list of node ids with -1 pads."""
    rem = dict(zip(nodes.tolist(), cnts.tolist()))
    ready = {int(n): 0 for n in nodes}            # node -> earliest col
    out = []
    t = 0
    while rem:
        best, bestc = -1, -1
        for n, r in rem.items():
            if ready[n] <= t and r > bestc:
                best, bestc = n, r
        if best < 0:
            out.append(-1)
        else:
            out.append(best)
            rem[best] -= 1
            if rem[best] == 0:
                del rem[best]
            # chunk boundaries are full barriers (Tile serializes calls),
            # so the cooldown never needs to cross one
            ready[best] = min(t + MINSP, ((t // CHUNK_COLS) + 1) * CHUNK_COLS)
        t += 1
    return out


def _prep(inputs):
    in_feat = np.asarray(inputs["in_feat"], np.float32)
    src = np.asarray(inputs["src"]).astype(np.int64)
    dst = np.asarray(inputs["dst"]).astype(np.int64)
    N, IN = in_feat.shape
    E = src.shape[0]
    assert N % NCORES == 0
    NLOC = N // NCORES
    NCOLS = -(-NLOC // 128)
    if NCOLS % 2:
        NCOLS += 1                      # NLOCP must be a multiple of 256
    if NCOLS * 128 <= NLOC:
        NCOLS += 2                      # spare tokens for the dummy dst
    NLOCP = NCOLS * 128
    assert NLOCP > NLOC
    SHARD = NLOCP                       # shard rows; [NLOC, NLOCP) are zeros
    TROWS = NCORES * SHARD
    WBASE = 2 * SHARD                   # window stride; zero rows at rel NLOC+
    assert WBASE <= 32768
    NW = -(-TROWS // WBASE)
    NGRP = NLOCP // 256
    DUMMY_TOK = NLOCP - 1
    PAD_REL = NLOC                      # a zero row of shard 2w, rel to base

    deg = np.bincount(dst, minlength=N).astype(np.int64)
    d_inv = (np.maximum(deg.astype(np.float32), 1.0) ** -0.5).astype(np.float32)

    order = np.argsort(-deg, kind="stable")      # rank -> orig node id
    core_of_rank = _snake(N, NCORES)
    local_of_rank = np.arange(N) // NCORES
    core = np.empty(N, np.int64)
    local = np.empty(N, np.int64)
    core[order] = core_of_rank
    local[order] = local_of_rank
    row = core * SHARD + local                   # table row of each orig node

    perm = np.empty((NCORES, NLOC), np.int64)    # perm[c][l] = orig node
    perm[core[order], local[order]] = order

    # ---- per (core, window) slot grids
    e_core = core[dst]
    s_row = row[src]
    e_w = s_row // WBASE
    s_rel = (s_row - e_w * WBASE).astype(np.int64)
    d_tok = local[dst]

    # first pass: per-core/window lane assignment + spacing-aware schedules.
    # Same dst must always land in the same lane (same DMA engine) and its
    # occurrences must be >= MINSP columns apart within a chunk (CCE RMW
    # hazard); pads fill the gaps.
    cols_cw = np.zeros((NCORES, NW), np.int64)
    lane_info = {}
    for c in range(NCORES):
        m_c = e_core == c
        for w in range(NW):
            m = m_c & (e_w == w)
            dt = d_tok[m]
            sr = s_rel[m]
            cnt = np.bincount(dt, minlength=NLOCP).astype(np.int64)
            nrank = np.argsort(-cnt, kind="stable")
            lane = np.empty(NLOCP, np.int64)
            lane[nrank] = _snake(NLOCP, 128)
            # per-lane schedule: greedy max-remaining with MINSP cooldown
            scheds = []
            for p in range(128):
                nodes = nrank[lane[nrank] == p]
                cnts = cnt[nodes]
                nodes = nodes[cnts > 0]
                cnts = cnt[nodes]
                scheds.append(_schedule_lane(nodes, cnts))
            cols_cw[c, w] = max((len(sc) for sc in scheds), default=0)
            lane_info[(c, w)] = (dt, sr, lane, scheds)
    cols_w = cols_cw.max(axis=0)                 # static per-window widths

    # chunk schedule (identical across cores): (window, ncols) pieces
    chunks = []
    for w in range(NW):
        rem = int(cols_w[w])
        while rem > 0:
            take = min(CHUNK_COLS, rem)
            chunks.append((w, take))
            rem -= take
    NCH = len(chunks)

    # second pass: fill slot arrays and wrap per chunk
    widx = np.zeros((NCORES, NCH, 128, IDXW), np.int16)
    sidx = np.zeros((NCORES, NCH, 128, IDXW), np.int16)
    for c in range(NCORES):
        for w in range(NW):
            dt, sr, lane, scheds = lane_info[(c, w)]
            S = int(cols_w[w]) * 128
            g_arr = np.full(S, PAD_REL, np.int64)
            s_arr = np.full(S, DUMMY_TOK, np.int64)
            if dt.size:
                # emitted (node, col) pairs per lane, grouped by node in
                # column order; edge k of node n -> n's k-th scheduled column
                em_node = []
                em_col = []
                em_lane = []
                for p in range(128):
                    sc = scheds[p]
                    for col, n in enumerate(sc):
                        if n >= 0:
                            em_node.append(n)
                            em_col.append(col)
                            em_lane.append(p)
                em_node = np.array(em_node, np.int64)
                em_col = np.array(em_col, np.int64)
                em_lane = np.array(em_lane, np.int64)
                oe = np.argsort(em_node, kind="stable")   # col order kept
                # edges sorted by node
                o = np.argsort(dt, kind="stable")
                assert em_node.size == dt.size
                g = em_col[oe] * 128 + em_lane[oe]
                g_arr[g] = sr[o]
                s_arr[g] = dt[o]
            # split into this window's chunks
            off = 0
            for k, (wk, nck) in enumerate(chunks):
                if wk != w:
                    continue
                size = nck * 128
                ga = g_arr[off:off + size]
                sa = s_arr[off:off + size]
                gw = ga.reshape(size // 16, 16).T.astype(np.int16)  # [16, s/16]
                sw = sa.reshape(size // 16, 16).T.astype(np.int16)
                widx[c, k, :, : size // 16] = np.tile(gw, (8, 1))
                sidx[c, k, :, : size // 16] = np.tile(sw, (8, 1))
                off += size

    # ---- folded weights (float64 for stability)
    W1 = np.asarray(inputs["W1"], np.float64)
    b1 = np.asarray(inputs["b1"], np.float64)
    W2 = np.asarray(inputs["W2"], np.float64)
    b2 = np.asarray(inputs["b2"], np.float64)
    W3 = np.asarray(inputs["W3"], np.float64)
    b3 = np.asarray(inputs["b3"], np.float64)
    W4 = np.asarray(inputs["W4"], np.float64)
    b4 = np.asarray(inputs["b4"], np.float64)
    lds = [np.asarray(inputs[f"ld{i+1}"], np.float64) for i in range(3)]
    cWs = [np.asarray(inputs[f"cW{i+1}"], np.float64) for i in range(3)]
    cbs = [np.asarray(inputs[f"cb{i+1}"], np.float64) for i in range(3)]

    K01 = np.zeros((H, H))
    KB1 = np.zeros((H, H))
    KB2 = np.zeros((H, H))
    b_emb = b3.copy()
    for i in range(3):
        th = THETAS[i]
        W3Ti = W3[:, i * H:(i + 1) * H].T          # [k, j]
        K01 += (th[1] + th[2]) * W3Ti
        K01 += th[0] * ((lds[i][0][:, None] * cWs[i].T) @ W3Ti)
        alpha = -th[1] * lds[i][1] - th[2] * (lds[i][1] + lds[i][2])
        beta = th[2] * lds[i][1] * lds[i][2]
        KB1 += alpha[:, None] * W3Ti
        KB2 += beta[:, None] * W3Ti
        b_emb += W3[:, i * H:(i + 1) * H] @ cbs[i]
    w4b = np.concatenate([W4.T, b4[None, :]], axis=0)        # [H+1, C]

    meta = dict(N=N, E=E, IN=IN, NLOC=NLOC, NCOLS=NCOLS, NLOCP=NLOCP,
                SHARD=SHARD, TROWS=TROWS, WBASE=WBASE, NW=NW, NGRP=NGRP,
                NCH=NCH, chunks=chunks)

    # ---- per-core input maps
    in_maps = []
    for c in range(NCORES):
        xin = np.zeros((IN, NLOCP), BF16NP)
        xin[:, :NLOC] = in_feat[perm[c]].T.astype(BF16NP)
        dinv = np.zeros((128, NCOLS, 1), np.float32)
        lidx = np.arange(NLOC)
        dinv[lidx % 128, lidx // 128, 0] = d_inv[perm[c]]
        in_maps.append({
            "xin": xin,
            "widx": widx[c].reshape(NCH * 128, IDXW),
            "sidx": sidx[c].reshape(NCH * 128, IDXW),
            "dinv": dinv,
            "w1t": W1.T.astype(BF16NP).copy(),
            "b1": b1.astype(np.float32).reshape(H, 1),
            "w2t": W2.T.astype(BF16NP).copy(),
            "b2": b2.astype(np.float32).reshape(H, 1),
            "k01": K01.astype(BF16NP),
            "kb1": KB1.astype(BF16NP),
            "kb2": KB2.astype(BF16NP),
            "bemb": b_emb.astype(np.float32).reshape(H, 1),
            "w4b": w4b.astype(BF16NP),
        })
    return meta, in_maps, perm


# ---------------------------------------------------------------- builder

def _build(meta):
    IN = meta["IN"]
    NLOC, NCOLS, NLOCP = meta["NLOC"], meta["NCOLS"], meta["NLOCP"]
    SHARD, TROWS, WBASE = meta["SHARD"], meta["TROWS"], meta["WBASE"]
    NGRP, NCH, chunks = meta["NGRP"], meta["NCH"], meta["chunks"]
    HC = H + C_OUT
    Relu = mybir.ActivationFunctionType.Relu

    nc = bacc.Bacc("TRN2", target_bir_lowering=False, debug=False,
                   num_devices=NCORES, num_swdge_queues=2)

    xin = nc.dram_tensor("xin", [IN, NLOCP], BF16, kind="ExternalInput")
    widx = nc.dram_tensor("widx", [NCH * 128, IDXW], I16, kind="ExternalInput")
    sidx = nc.dram_tensor("sidx", [NCH * 128, IDXW], I16, kind="ExternalInput")
    dinv = nc.dram_tensor("dinv", [128, NCOLS, 1], F32, kind="ExternalInput")
    w1t = nc.dram_tensor("w1t", [IN, H], BF16, kind="ExternalInput")
    b1 = nc.dram_tensor("b1", [H, 1], F32, kind="ExternalInput")
    w2t = nc.dram_tensor("w2t", [H, H], BF16, kind="ExternalInput")
    b2 = nc.dram_tensor("b2", [H, 1], F32, kind="ExternalInput")
    k01 = nc.dram_tensor("k01", [H, H], BF16, kind="ExternalInput")
    kb1 = nc.dram_tensor("kb1", [H, H], BF16, kind="ExternalInput")
    kb2 = nc.dram_tensor("kb2", [H, H], BF16, kind="ExternalInput")
    bemb = nc.dram_tensor("bemb", [H, 1], F32, kind="ExternalInput")
    w4b = nc.dram_tensor("w4b", [H + 1, C_OUT], BF16, kind="ExternalInput")
    out = nc.dram_tensor("out", [NLOCP, HC], F32, kind="ExternalOutput")


    with tile.TileContext(nc) as tc:
        with tc.tile_pool(name="dram", bufs=1, space="DRAM") as dram, \
             tc.tile_pool(name="cst", bufs=1) as cst, \
             tc.tile_pool(name="sb", bufs=1) as sb, \
             tc.tile_pool(name="msgp", bufs=2) as msgp, \
             tc.tile_pool(name="idxp", bufs=2) as idxp, \
             tc.tile_pool(name="midp", bufs=2) as midp, \
             tc.tile_pool(name="ps", bufs=6, space="PSUM") as ps:

            table1 = dram.tile([TROWS, H], F32, addr_space="Shared")
            table2 = dram.tile([TROWS, H], F32, addr_space="Shared")
            x1s = dram.tile([SHARD, H], F32)
            x2s = dram.tile([SHARD, H], F32)

            # ---------- constants
            w1t_t = cst.tile([IN, H], BF16)
            nc.sync.dma_start(w1t_t[:], w1t[:])
            w2t_t = cst.tile([H, H], BF16)
            nc.sync.dma_start(w2t_t[:], w2t[:])
            b1_t = cst.tile([H, 1], F32)
            nc.sync.dma_start(b1_t[:], b1[:])
            b2_t = cst.tile([H, 1], F32)
            nc.sync.dma_start(b2_t[:], b2[:])
            k01_t = cst.tile([H, H], BF16)
            nc.sync.dma_start(k01_t[:], k01[:])
            kb1_t = cst.tile([H, H], BF16)
            nc.sync.dma_start(kb1_t[:], kb1[:])
            kb2_t = cst.tile([H, H], BF16)
            nc.sync.dma_start(kb2_t[:], kb2[:])
            bemb_t = cst.tile([H, 1], F32)
            nc.sync.dma_start(bemb_t[:], bemb[:])
            w4b_t = cst.tile([H + 1, C_OUT], BF16)
            nc.sync.dma_start(w4b_t[:], w4b[:])
            dinv_t = cst.tile([128, NCOLS, 1], F32)
            nc.sync.dma_start(dinv_t[:], dinv[:])
            d2_t = cst.tile([128, NCOLS, 1], F32)
            nc.vector.tensor_mul(d2_t[:], dinv_t[:], dinv_t[:])
            i64 = cst.tile([H, H], BF16)
            make_identity(nc, i64[:])
            i128 = cst.tile([128, 128], BF16)
            make_identity(nc, i128[:])

            # ---------- MLP (transposed chain)
            xin_t = sb.tile([IN, NLOCP], BF16, tag="bigA")
            nc.sync.dma_start(xin_t[:], xin[:])
            h2T = sb.tile([H, NLOCP], BF16, tag="h2T")
            for s0 in range(0, NLOCP, 512):
                cw = min(512, NLOCP - s0)
                sl = slice(s0, s0 + cw)
                p1 = ps.tile([H, 512], F32, tag="ps")
                nc.tensor.matmul(p1[:, :cw], lhsT=w1t_t[:], rhs=xin_t[:, sl],
                                 start=True, stop=True)
                ht = midp.tile([H, 512], BF16, tag="ht")
                nc.scalar.activation(ht[:, :cw], p1[:, :cw], Relu, bias=b1_t[:])
                p2 = ps.tile([H, 512], F32, tag="ps")
                nc.tensor.matmul(p2[:, :cw], lhsT=w2t_t[:], rhs=ht[:, :cw],
                                 start=True, stop=True)
                nc.scalar.activation(h2T[:, sl], p2[:, :cw], Relu, bias=b2_t[:])

            # ---------- x1 = h * d (node-major), staged then DMA'd out
            x1_sb = sb.tile([128, NCOLS, H], F32, tag="xsb")
            for g0 in range(0, NCOLS, 7):
                gn = min(7, NCOLS - g0)
                px = ps.tile([128, 7 * H], F32, tag="ps")
                for t in range(gn):
                    col = g0 + t
                    nc.tensor.matmul(px[:, t * H:(t + 1) * H],
                                     lhsT=h2T[:, col * 128:(col + 1) * 128],
                                     rhs=i64[:], start=True, stop=True)
                nc.vector.tensor_mul(
                    x1_sb[:, g0:g0 + gn, :],
                    px[:, :gn * H].rearrange("p (c h) -> p c h", h=H),
                    dinv_t[:, g0:g0 + gn, :].to_broadcast([128, gn, H]))
            nc.sync.dma_start(
                x1s[:].rearrange("(c p) h -> p c h", p=128), x1_sb[:])
            nc.gpsimd.collective_compute(
                "AllGather", mybir.AluOpType.bypass,
                replica_groups=[list(range(NCORES))],
                ins=[x1s[:].opt()], outs=[table1[:].opt()])

            # ---------- aggregation steps
            def agg_step(own_tag, peer_tag, table):
                own = sb.tile([128, NGRP, H], F32, tag=own_tag, name=own_tag)
                peer = sb.tile([128, NGRP, H], F32, tag=peer_tag, name=peer_tag)
                nc.vector.memset(own[:], 0.0)
                nc.vector.memset(peer[:], 0.0)
                for k, (w, nck) in enumerate(chunks):
                    size = nck * 128
                    gi = idxp.tile([128, IDXW], I16, tag="gi", name="gi")
                    nc.sync.dma_start(gi[:], widx[k * 128:(k + 1) * 128, :])
                    si = idxp.tile([128, IDXW], I16, tag="si", name="si")
                    nc.sync.dma_start(si[:], sidx[k * 128:(k + 1) * 128, :])
                    msg = msgp.tile([128, CHUNK_COLS, H], F32, tag="msg",
                                    name="msg")
                    nc.gpsimd.dma_gather(
                        out_ap=msg[:, :nck, :],
                        in_ap=table[w * WBASE:, :],
                        idxs_ap=gi[:, : size // 16],
                        num_idxs=size, num_idxs_reg=size,
                        elem_size=H, queue_num=1,
                        single_packet=False)
                    nc.gpsimd.dma_scatter_add(
                        out_ap=own[:],
                        in_ap=msg[:, :nck, :],
                        idxs_ap=si[:, : size // 16],
                        num_idxs=size, num_idxs_reg=size,
                        elem_size=H,
                        sbuf_tokens_per_rank=128,
                        parity_reg=0,
                        out_ap_other=peer[:],
                        queue_num=0, single_packet=False)
                return own, peer

            def par_view(ap3, par):
                # [128, NCOLS, X] -> the even/odd columns [128, NGRP, X]
                v = ap3.rearrange("p (c t) x -> p c t x", t=2)
                return v[:, :, par:par + 1, :].rearrange("p c o x -> p c (o x)")

            own1, peer1 = agg_step("accA", "accB", table1)

            # x2 = M1 * d^2  (M1 even cols in own1, odd cols in peer1)
            x2_sb = sb.tile([128, NCOLS, H], F32, tag="xsb")
            for par, acc in ((0, own1), (1, peer1)):
                nc.vector.tensor_mul(
                    par_view(x2_sb[:], par), acc[:],
                    par_view(d2_t[:], par).to_broadcast([128, NGRP, H]))
            nc.sync.dma_start(
                x2s[:].rearrange("(c p) h -> p c h", p=128), x2_sb[:])
            nc.gpsimd.collective_compute(
                "AllGather", mybir.AluOpType.bypass,
                replica_groups=[list(range(NCORES))],
                ins=[x2s[:].opt()], outs=[table2[:].opt()])

            # B1 = M1 * d (node-major bf16; transposed on demand at mm4)
            b1a = sb.tile([128, NGRP, H], BF16, tag="b1a", name="b1a")
            b1b = sb.tile([128, NGRP, H], BF16, tag="b1b", name="b1b")
            for par, acc, dst in ((0, own1, b1a), (1, peer1, b1b)):
                nc.vector.tensor_mul(
                    dst[:], acc[:],
                    par_view(dinv_t[:], par).to_broadcast([128, NGRP, H]))

            own2, peer2 = agg_step("accA", "accB", table2)
            b2a = sb.tile([128, NGRP, H], BF16, tag="b2a", name="b2a")
            b2b = sb.tile([128, NGRP, H], BF16, tag="b2b", name="b2b")
            for par, acc, dst in ((0, own2, b2a), (1, peer2, b2b)):
                nc.vector.tensor_mul(
                    dst[:], acc[:],
                    par_view(dinv_t[:], par).to_broadcast([128, NGRP, H]))

            # ---------- emb (transposed) with on-demand B transposes
            embT = sb.tile([H + 1, NLOCP], BF16, tag="xsb")
            nc.vector.memset(embT[H:H + 1, :], 1.0)
            for s0 in range(0, NLOCP, 512):
                cw = min(512, NLOCP - s0)
                ncols_here = cw // 128
                b1T = midp.tile([H, 512], BF16, tag="b1T", name="b1T")
                b2T = midp.tile([H, 512], BF16, tag="b2T", name="b2T")
                pt1 = ps.tile([H, 512], BF16, tag="ps")
                pt2 = ps.tile([H, 512], BF16, tag="ps")
                for t in range(ncols_here):
                    col = s0 // 128 + t
                    srcs = (b1a if col % 2 == 0 else b1b,
                            b2a if col % 2 == 0 else b2b)
                    g2 = col // 2
                    nc.tensor.transpose(
                        pt1[:, t * 128:(t + 1) * 128],
                        srcs[0][:, g2:g2 + 1, :].rearrange("p o h -> p (o h)"),
                        i128[:])
                    nc.tensor.transpose(
                        pt2[:, t * 128:(t + 1) * 128],
                        srcs[1][:, g2:g2 + 1, :].rearrange("p o h -> p (o h)"),
                        i128[:])
                nc.vector.tensor_copy(b1T[:, :cw], pt1[:, :cw])
                nc.vector.tensor_copy(b2T[:, :cw], pt2[:, :cw])
                pe = ps.tile([H, 512], F32, tag="ps")
                sl = slice(s0, s0 + cw)
                nc.tensor.matmul(pe[:, :cw], lhsT=k01_t[:], rhs=h2T[:, sl],
                                 start=True, stop=False)
                nc.tensor.matmul(pe[:, :cw], lhsT=kb1_t[:], rhs=b1T[:, :cw],
                                 start=False, stop=False)
                nc.tensor.matmul(pe[:, :cw], lhsT=kb2_t[:], rhs=b2T[:, :cw],
                                 start=False, stop=True)
                nc.scalar.activation(embT[0:H, sl], pe[:, :cw], Relu,
                                     bias=bemb_t[:])

            # ---------- outputs: emb node-major + logits, packed [NLOC, 66]
            outb = sb.tile([128, NCOLS, HC], F32, tag="bigA")
            for g0 in range(0, NCOLS, 7):
                gn = min(7, NCOLS - g0)
                po = ps.tile([128, 7 * HC], F32, tag="ps")
                for t in range(gn):
                    col = g0 + t
                    tsl = slice(col * 128, (col + 1) * 128)
                    off = t * HC
                    nc.tensor.matmul(po[:, off:off + H],
                                     lhsT=embT[0:H, tsl], rhs=i64[:],
                                     start=True, stop=True)
                    nc.tensor.matmul(po[:, off + H:off + HC],
                                     lhsT=embT[:, tsl], rhs=w4b_t[:],
                                     start=True, stop=True)
                nc.vector.tensor_copy(
                    outb[:, g0:g0 + gn, :],
                    po[:, :gn * HC].rearrange("p (c h) -> p c h", h=HC))
            nc.sync.dma_start(
                out[:].rearrange("(c p) h -> p c h", p=128), outb[:])

    nc.compile()
    return nc


# ---------------------------------------------------------------- entry

def run(inputs, trace=False, trace_kwargs=None):
    meta, in_maps, perm = _prep(inputs)
    nc = _build(meta)
    kw = {}
    if trace:
        kw["trace"] = True
        if trace_kwargs:
            kw["trace_kwargs"] = trace_kwargs
    res = run_bass_kernel_spmd(nc, in_maps, core_ids=list(range(NCORES)), **kw)
    N = meta["N"]
    logits = np.zeros((N, C_OUT), np.float32)
    emb = np.zeros((N, H), np.float32)
    NLOC = meta["NLOC"]
    for c in range(NCORES):
        o = np.asarray(res.results[c]["out"])[:NLOC]
        emb[perm[c]] = o[:, :H]
        logits[perm[c]] = o[:, H:]
    return (logits, emb), res


def kernel(**inputs):
    (logits, emb), _ = run(inputs)
    return logits, emb


# revision 12
# speedup vs baseline: 1.1964x; 1.1964x over previous
"""AdaGNN (gnn_message_passing) distributed Bass kernel for 8 TRN2 NeuronCores.

Math refactoring (exact, up to fp reassociation):
  The reference runs 3 PolyConvs, each applying the unnormalized Laplacian
  twice (6 gather+segment_sum rounds).  All 3 convs start from the same h and
  the per-feature diagonal scales (ld) commute through the adjacency A, so
  only TWO aggregations are needed:
     M1 = A @ (h * d)          (d = deg^-1/2 per node)
     M2 = A @ (M1 * d^2)
  With B1 = M1*d, B2 = M2*d every conv output is
     h_i = th0*(h*ld_i0)@cW_i^T + cb_i + c_h(i)*h + B1*alpha_i + B2*beta_i
  and emb = relu(concat_i(h_i) @ W3^T + b3) collapses to
     emb = relu(h @ K01 + B1 @ KB1 + B2 @ KB2 + b_emb)
  with K01/KB1/KB2/b_emb folded on the host from the (tiny) parameters.

Distribution: nodes are degree-sorted and snake-assigned to the 8 cores
(dst ownership).  Each core computes its h/x shard, AllGathers the x table
([8*(NLOC+1), 64] f32, one zero row per shard), then gathers per-edge rows
with dma_gather (4 int16 windows, per-window lane-balanced slot grids, pads
point at the window's zero row) and segment-sums with dma_scatter_add's
SBUF-parity CCE accumulate (same dst always in the same lane -> same DMA
engine -> no RMW race; consecutive scatter chunks are serialized by Tile).
"""

import numpy as np
import ml_dtypes

import concourse.bass as bass
import concourse.mybir as mybir
import concourse.tile as tile
import concourse.bacc as bacc
from concourse.bass_utils import run_bass_kernel_spmd
from concourse.masks import make_identity

F32 = mybir.dt.float32
BF16 = mybir.dt.bfloat16
I16 = mybir.dt.int16
BF16NP = ml_dtypes.bfloat16

NCORES = 8
H = 64
C_OUT = 2
THETAS = ((3.0, -3.0, 0.75), (0.0, 3.0, -1.5), (0.0, 0.0, 0.75))
GCH_COLS = 96                        # gather chunk columns (<=1009 ring descs)
SCH_POS = 49                         # scatter chunk positions (all tokens unique)
IDXW = GCH_COLS * 128 // 16          # free dim of a wrapped gather idx tile
SIDXW = SCH_POS * 128 // 16


# ---------------------------------------------------------------- host prep

def _snake(n_items, n_bins):
    """rank -> bin, serpentine."""
    r = np.arange(n_items)
    blk, pos = r // n_bins, r % n_bins
    return np.where(blk % 2 == 0, pos, n_bins - 1 - pos)


def _prep(inputs):
    in_feat = np.asarray(inputs["in_feat"], np.float32)
    src = np.asarray(inputs["src"]).astype(np.int64)
    dst = np.asarray(inputs["dst"]).astype(np.int64)
    N, IN = in_feat.shape
    E = src.shape[0]
    assert N % NCORES == 0
    NLOC = N // NCORES
    NCOLS = -(-NLOC // 128)
    if NCOLS % 2:
        NCOLS += 1                      # NLOCP must be a multiple of 256
    if NCOLS * 128 <= NLOC:
        NCOLS += 2                      # spare tokens for the dummy dst
    NLOCP = NCOLS * 128
    assert NLOCP > NLOC
    SHARD = NLOCP                       # shard rows; [NLOC, NLOCP) are zeros
    TROWS = NCORES * SHARD
    WBASE = 2 * SHARD                   # window stride; zero rows at rel NLOC+
    assert WBASE <= 32768
    NW = -(-TROWS // WBASE)
    NGRP = NLOCP // 256
    DUMMY_TOK = NLOCP - 1
    PAD_REL = NLOC                      # a zero row of shard 2w, rel to base

    deg = np.bincount(dst, minlength=N).astype(np.int64)
    d_inv = (np.maximum(deg.astype(np.float32), 1.0) ** -0.5).astype(np.float32)

    order = np.argsort(-deg, kind="stable")      # rank -> orig node id
    core_of_rank = _snake(N, NCORES)
    local_of_rank = np.arange(N) // NCORES
    core = np.empty(N, np.int64)
    local = np.empty(N, np.int64)
    core[order] = core_of_rank
    local[order] = local_of_rank
    row = core * SHARD + local                   # table row of each orig node

    perm = np.empty((NCORES, NLOC), np.int64)    # perm[c][l] = orig node
    perm[core[order], local[order]] = order

    # ---- per (core, window) slot grids
    e_core = core[dst]
    s_row = row[src]
    e_w = s_row // WBASE
    s_rel = (s_row - e_w * WBASE).astype(np.int64)
    d_tok = local[dst]

    # Per (core, window): sort nodes by window in-degree (descending);
    # window-rank r sits at grid cell (partition r%128, position r//128).
    # Position k gets D[k] slot columns (max over cores of the position's
    # largest degree); a node's edges fill its cell's columns, pads point
    # at a zero table row.  A DVE segmented reduce turns the gathered grid
    # into per-node partials (window-rank order), and one dma_scatter_add
    # per ~SCH_POS positions routes them into the canonical accumulators
    # (every token distinct within a call -> no CCE RMW hazards).
    per_cw = {}
    deg_cw = np.zeros((NCORES, NW, NLOCP), np.int64)
    for c in range(NCORES):
        m_c = e_core == c
        for w in range(NW):
            m = m_c & (e_w == w)
            dt = d_tok[m]
            sr = s_rel[m]
            cnt = np.bincount(dt, minlength=NLOCP).astype(np.int64)
            deg_cw[c, w] = cnt
            nrank = np.argsort(-cnt, kind="stable")      # wrank -> node
            per_cw[(c, w)] = (dt, sr, cnt, nrank)

    NPOS = NLOCP // 128
    D_w = np.zeros((NW, NPOS), np.int64)
    for w in range(NW):
        for c in range(NCORES):
            _, _, cnt, nrank = per_cw[(c, w)]
            dsorted = cnt[nrank]
            D_w[w] = np.maximum(D_w[w], dsorted[0::128][:NPOS])
    npos_w = [int((D_w[w] > 0).sum()) for w in range(NW)]

    # gather chunks: (window, [(pos0, npos, D)]) runs packed to <=GCH_COLS
    gchunks = []
    for w in range(NW):
        runs = []          # maximal equal-D runs
        k = 0
        while k < npos_w[w]:
            j = k
            while j < npos_w[w] and D_w[w][j] == D_w[w][k]:
                j += 1
            runs.append((k, j - k, int(D_w[w][k])))
            k = j
        cur, cw = [], 0
        for (k0, nk, D) in runs:
            while nk > 0:
                fit = min(nk, max(0, (GCH_COLS - cw) // D))
                if fit == 0:
                    gchunks.append((w, cur, cw))
                    cur, cw = [], 0
                    continue
                cur.append((k0, fit, D))
                cw += fit * D
                k0 += fit
                nk -= fit
        if cur:
            gchunks.append((w, cur, cw))
    # scatter chunks: (window, pos0, npos)
    schunks = []
    for w in range(NW):
        k = 0
        while k < npos_w[w]:
            take = min(SCH_POS, npos_w[w] - k)
            schunks.append((w, k, take))
            k += take
    NGC, NSC = len(gchunks), len(schunks)

    def wrap_rep(a):
        return np.tile(a.reshape(-1, 16).T, (8, 1)).astype(np.int16)

    widx = np.zeros((NCORES, NGC, 128, IDXW), np.int16)
    sidx = np.zeros((NCORES, NSC, 128, SIDXW), np.int16)
    for c in range(NCORES):
        for w in range(NW):
            dt, sr, cnt, nrank = per_cw[(c, w)]
            wr = np.empty(NLOCP, np.int64)
            wr[nrank] = np.arange(NLOCP)
            # column offset of each position in the window grid
            C_k = np.concatenate(([0], np.cumsum(D_w[w])[:-1]))
            cols_total = int(np.cumsum(D_w[w])[-1])
            g_arr = np.full(cols_total * 128, PAD_REL, np.int64)
            if dt.size:
                r = wr[dt]
                o = np.lexsort((np.arange(dt.size), r))
                r_s = r[o]
                starts = np.searchsorted(r_s, np.arange(NLOCP))
                j = np.arange(r_s.size) - starts[r_s]
                col = C_k[r_s // 128] + j
                g = col * 128 + (r_s % 128)
                g_arr[g] = sr[o]
            # slice by gather chunks of this window
            for k, (wk, runs, cw) in enumerate(gchunks):
                if wk != w:
                    continue
                parts = []
                for (k0, nk, D) in runs:
                    c0 = int(C_k[k0])
                    parts.append(g_arr[c0 * 128:(c0 + nk * D) * 128])
                ga = np.concatenate(parts) if parts else np.zeros(0, np.int64)
                assert ga.size == cw * 128
                widx[c, k, :, : ga.size // 16] = wrap_rep(ga)
            for k, (wk, p0, npos) in enumerate(schunks):
                if wk != w:
                    continue
                toks = nrank[p0 * 128:(p0 + npos) * 128]
                toks = np.where(toks < NLOC, toks, DUMMY_TOK)
                # stream position g = j*128 + p  <->  wrank (p0+j)*128 + p
                sa = toks.reshape(npos, 128).ravel()
                sidx[c, k, :, : sa.size // 16] = wrap_rep(sa)

    # ---- folded weights (float64 for stability)
    W1 = np.asarray(inputs["W1"], np.float64)
    b1 = np.asarray(inputs["b1"], np.float64)
    W2 = np.asarray(inputs["W2"], np.float64)
    b2 = np.asarray(inputs["b2"], np.float64)
    W3 = np.asarray(inputs["W3"], np.float64)
    b3 = np.asarray(inputs["b3"], np.float64)
    W4 = np.asarray(inputs["W4"], np.float64)
    b4 = np.asarray(inputs["b4"], np.float64)
    lds = [np.asarray(inputs[f"ld{i+1}"], np.float64) for i in range(3)]
    cWs = [np.asarray(inputs[f"cW{i+1}"], np.float64) for i in range(3)]
    cbs = [np.asarray(inputs[f"cb{i+1}"], np.float64) for i in range(3)]

    K01 = np.zeros((H, H))
    KB1 = np.zeros((H, H))
    KB2 = np.zeros((H, H))
    b_emb = b3.copy()
    for i in range(3):
        th = THETAS[i]
        W3Ti = W3[:, i * H:(i + 1) * H].T          # [k, j]
        K01 += (th[1] + th[2]) * W3Ti
        K01 += th[0] * ((lds[i][0][:, None] * cWs[i].T) @ W3Ti)
        alpha = -th[1] * lds[i][1] - th[2] * (lds[i][1] + lds[i][2])
        beta = th[2] * lds[i][1] * lds[i][2]
        KB1 += alpha[:, None] * W3Ti
        KB2 += beta[:, None] * W3Ti
        b_emb += W3[:, i * H:(i + 1) * H] @ cbs[i]
    w4b = np.concatenate([W4.T, b4[None, :]], axis=0)        # [H+1, C]

    meta = dict(N=N, E=E, IN=IN, NLOC=NLOC, NCOLS=NCOLS, NLOCP=NLOCP,
                SHARD=SHARD, TROWS=TROWS, WBASE=WBASE, NW=NW, NGRP=NGRP,
                NGC=NGC, NSC=NSC, gchunks=gchunks, schunks=schunks,
                NPOS=NPOS)

    # ---- per-core input maps
    in_maps = []
    for c in range(NCORES):
        xin = np.zeros((IN, NLOCP), BF16NP)
        xin[:, :NLOC] = in_feat[perm[c]].T.astype(BF16NP)
        dinv = np.zeros((128, NCOLS, 1), np.float32)
        lidx = np.arange(NLOC)
        dinv[lidx % 128, lidx // 128, 0] = d_inv[perm[c]]
        in_maps.append({
            "xin": xin,
            "widx": widx[c].reshape(NGC * 128, IDXW),
            "sidx": sidx[c].reshape(NSC * 128, SIDXW),
            "dinv": dinv,
            "w1t": W1.T.astype(BF16NP).copy(),
            "b1": b1.astype(np.float32).reshape(H, 1),
            "w2t": W2.T.astype(BF16NP).copy(),
            "b2": b2.astype(np.float32).reshape(H, 1),
            "k01": K01.astype(BF16NP),
            "kb1": KB1.astype(BF16NP),
            "kb2": KB2.astype(BF16NP),
            "bemb": b_emb.astype(np.float32).reshape(H, 1),
            "w4b": w4b.astype(BF16NP),
        })
    return meta, in_maps, perm


# ---------------------------------------------------------------- builder

def _build(meta):
    IN = meta["IN"]
    NLOC, NCOLS, NLOCP = meta["NLOC"], meta["NCOLS"], meta["NLOCP"]
    SHARD, TROWS, WBASE = meta["SHARD"], meta["TROWS"], meta["WBASE"]
    NGRP, NGC, NSC = meta["NGRP"], meta["NGC"], meta["NSC"]
    gchunks, schunks = meta["gchunks"], meta["schunks"]
    HC = H + C_OUT
    Relu = mybir.ActivationFunctionType.Relu

    nc = bacc.Bacc("TRN2", target_bir_lowering=False, debug=False,
                   num_devices=NCORES, num_swdge_queues=2)

    xin = nc.dram_tensor("xin", [IN, NLOCP], BF16, kind="ExternalInput")
    widx = nc.dram_tensor("widx", [NGC * 128, IDXW], I16, kind="ExternalInput")
    sidx = nc.dram_tensor("sidx", [NSC * 128, SIDXW], I16, kind="ExternalInput")
    dinv = nc.dram_tensor("dinv", [128, NCOLS, 1], F32, kind="ExternalInput")
    w1t = nc.dram_tensor("w1t", [IN, H], BF16, kind="ExternalInput")
    b1 = nc.dram_tensor("b1", [H, 1], F32, kind="ExternalInput")
    w2t = nc.dram_tensor("w2t", [H, H], BF16, kind="ExternalInput")
    b2 = nc.dram_tensor("b2", [H, 1], F32, kind="ExternalInput")
    k01 = nc.dram_tensor("k01", [H, H], BF16, kind="ExternalInput")
    kb1 = nc.dram_tensor("kb1", [H, H], BF16, kind="ExternalInput")
    kb2 = nc.dram_tensor("kb2", [H, H], BF16, kind="ExternalInput")
    bemb = nc.dram_tensor("bemb", [H, 1], F32, kind="ExternalInput")
    w4b = nc.dram_tensor("w4b", [H + 1, C_OUT], BF16, kind="ExternalInput")
    out = nc.dram_tensor("out", [NLOCP, HC], F32, kind="ExternalOutput")


    with tile.TileContext(nc) as tc:
        with tc.tile_pool(name="dram", bufs=1, space="DRAM") as dram, \
             tc.tile_pool(name="cst", bufs=1) as cst, \
             tc.tile_pool(name="sb", bufs=1) as sb, \
             tc.tile_pool(name="prtp", bufs=2) as prtp, \
             tc.tile_pool(name="idxp", bufs=2) as idxp, \
             tc.tile_pool(name="midp", bufs=2) as midp, \
             tc.tile_pool(name="ps", bufs=6, space="PSUM") as ps:

            table1 = dram.tile([TROWS, H], F32, addr_space="Shared")
            table2 = dram.tile([TROWS, H], F32, addr_space="Shared")
            x1s = dram.tile([SHARD, H], F32)
            x2s = dram.tile([SHARD, H], F32)

            # ---------- constants
            w1t_t = cst.tile([IN, H], BF16)
            nc.sync.dma_start(w1t_t[:], w1t[:])
            w2t_t = cst.tile([H, H], BF16)
            nc.sync.dma_start(w2t_t[:], w2t[:])
            b1_t = cst.tile([H, 1], F32)
            nc.sync.dma_start(b1_t[:], b1[:])
            b2_t = cst.tile([H, 1], F32)
            nc.sync.dma_start(b2_t[:], b2[:])
            k01_t = cst.tile([H, H], BF16)
            nc.sync.dma_start(k01_t[:], k01[:])
            kb1_t = cst.tile([H, H], BF16)
            nc.sync.dma_start(kb1_t[:], kb1[:])
            kb2_t = cst.tile([H, H], BF16)
            nc.sync.dma_start(kb2_t[:], kb2[:])
            bemb_t = cst.tile([H, 1], F32)
            nc.sync.dma_start(bemb_t[:], bemb[:])
            w4b_t = cst.tile([H + 1, C_OUT], BF16)
            nc.sync.dma_start(w4b_t[:], w4b[:])
            dinv_t = cst.tile([128, NCOLS, 1], F32)
            nc.sync.dma_start(dinv_t[:], dinv[:])
            d2_t = cst.tile([128, NCOLS, 1], F32)
            nc.vector.tensor_mul(d2_t[:], dinv_t[:], dinv_t[:])
            i64 = cst.tile([H, H], BF16)
            make_identity(nc, i64[:])
            i128 = cst.tile([128, 128], BF16)
            make_identity(nc, i128[:])

            # ---------- MLP (transposed chain)
            xin_t = sb.tile([IN, NLOCP], BF16, tag="bigA", bufs=2)
            nc.sync.dma_start(xin_t[:], xin[:])
            h2T = sb.tile([H, NLOCP], BF16, tag="h2T")
            for s0 in range(0, NLOCP, 512):
                cw = min(512, NLOCP - s0)
                sl = slice(s0, s0 + cw)
                p1 = ps.tile([H, 512], F32, tag="ps")
                nc.tensor.matmul(p1[:, :cw], lhsT=w1t_t[:], rhs=xin_t[:, sl],
                                 start=True, stop=True)
                ht = midp.tile([H, 512], BF16, tag="ht")
                nc.scalar.activation(ht[:, :cw], p1[:, :cw], Relu, bias=b1_t[:])
                p2 = ps.tile([H, 512], F32, tag="ps")
                nc.tensor.matmul(p2[:, :cw], lhsT=w2t_t[:], rhs=ht[:, :cw],
                                 start=True, stop=True)
                nc.scalar.activation(h2T[:, sl], p2[:, :cw], Relu, bias=b2_t[:])

            # ---------- x1 = h * d (node-major), staged then DMA'd out
            x1_sb = sb.tile([128, NCOLS, H], F32, tag="xsb")
            for g0 in range(0, NCOLS, 7):
                gn = min(7, NCOLS - g0)
                px = ps.tile([128, 7 * H], F32, tag="ps")
                for t in range(gn):
                    col = g0 + t
                    nc.tensor.matmul(px[:, t * H:(t + 1) * H],
                                     lhsT=h2T[:, col * 128:(col + 1) * 128],
                                     rhs=i64[:], start=True, stop=True)
                nc.vector.tensor_mul(
                    x1_sb[:, g0:g0 + gn, :],
                    px[:, :gn * H].rearrange("p (c h) -> p c h", h=H),
                    dinv_t[:, g0:g0 + gn, :].to_broadcast([128, gn, H]))
            nc.sync.dma_start(
                x1s[:].rearrange("(c p) h -> p c h", p=128), x1_sb[:])
            nc.gpsimd.collective_compute(
                "AllGather", mybir.AluOpType.bypass,
                replica_groups=[list(range(NCORES))],
                ins=[x1s[:].opt()], outs=[table1[:].opt()])

            # ---------- aggregation steps
            def agg_step(own_tag, peer_tag, table):
                own = sb.tile([128, NGRP, H], F32, tag=own_tag, name=own_tag)
                peer = sb.tile([128, NGRP, H], F32, tag=peer_tag, name=peer_tag)
                nc.vector.memset(own[:], 0.0)
                nc.vector.memset(peer[:], 0.0)
                # per-window partial sums in window-rank order
                parts = {}
                gi_by_w = {}
                for k, (w, runs, cw) in enumerate(gchunks):
                    size = cw * 128
                    gi = idxp.tile([128, IDXW], I16, tag="gi", name="gi")
                    nc.sync.dma_start(gi[:], widx[k * 128:(k + 1) * 128, :])
                    msg = sb.tile([128, GCH_COLS, H], F32, tag="bigA",
                                    name="msg", bufs=2)
                    nc.gpsimd.dma_gather(
                        out_ap=msg[:, :cw, :],
                        in_ap=table[w * WBASE:, :],
                        idxs_ap=gi[:, : size // 16],
                        num_idxs=size, num_idxs_reg=size,
                        elem_size=H, queue_num=1,
                        single_packet=False)
                    # segmented reduce: each equal-D run in one DVE op
                    c_off = 0
                    for (k0, nk, D) in runs:
                        sc_idx = k0 // SCH_POS
                        part = parts.get((w, sc_idx))
                        if part is None:
                            part = prtp.tile([128, SCH_POS, H], F32,
                                             tag="part", name="part")
                            parts[(w, sc_idx)] = part
                        # run may straddle a scatter-chunk boundary
                        kk0, nnk = k0, nk
                        while nnk > 0:
                            sci = kk0 // SCH_POS
                            pp = parts.get((w, sci))
                            if pp is None:
                                pp = prtp.tile([128, SCH_POS, H], F32,
                                               tag="part", name="part")
                                parts[(w, sci)] = pp
                            take = min(nnk, (sci + 1) * SCH_POS - kk0)
                            src = msg[:, c_off:c_off + take * D, :].rearrange(
                                "p (k d) h -> p k h d", d=D)
                            nc.vector.tensor_reduce(
                                pp[:, kk0 - sci * SCH_POS:
                                   kk0 - sci * SCH_POS + take, :],
                                src, axis=mybir.AxisListType.X,
                                op=mybir.AluOpType.add)
                            c_off += take * D
                            kk0 += take
                            nnk -= take
                for k, (w, p0, npos) in enumerate(schunks):
                    size = npos * 128
                    si = idxp.tile([128, SIDXW], I16, tag="si", name="si")
                    nc.sync.dma_start(si[:], sidx[k * 128:(k + 1) * 128, :])
                    part = parts[(w, p0 // SCH_POS)]
                    nc.gpsimd.dma_scatter_add(
                        out_ap=own[:],
                        in_ap=part[:, :npos, :],
                        idxs_ap=si[:, : size // 16],
                        num_idxs=size, num_idxs_reg=size,
                        elem_size=H,
                        sbuf_tokens_per_rank=128,
                        parity_reg=0,
                        out_ap_other=peer[:],
                        queue_num=0, single_packet=False)
                return own, peer

            def par_view(ap3, par):
                # [128, NCOLS, X] -> the even/odd columns [128, NGRP, X]
                v = ap3.rearrange("p (c t) x -> p c t x", t=2)
                return v[:, :, par:par + 1, :].rearrange("p c o x -> p c (o x)")

            own1, peer1 = agg_step("accA", "accB", table1)

            # x2 = M1 * d^2  (M1 even cols in own1, odd cols in peer1)
            x2_sb = sb.tile([128, NCOLS, H], F32, tag="xsb")
            for par, acc in ((0, own1), (1, peer1)):
                nc.vector.tensor_mul(
                    par_view(x2_sb[:], par), acc[:],
                    par_view(d2_t[:], par).to_broadcast([128, NGRP, H]))
            nc.sync.dma_start(
                x2s[:].rearrange("(c p) h -> p c h", p=128), x2_sb[:])
            nc.gpsimd.collective_compute(
                "AllGather", mybir.AluOpType.bypass,
                replica_groups=[list(range(NCORES))],
                ins=[x2s[:].opt()], outs=[table2[:].opt()])

            # B1 = M1 * d (node-major bf16; transposed on demand at mm4)
            b1a = sb.tile([128, NGRP, H], BF16, tag="b1a", name="b1a")
            b1b = sb.tile([128, NGRP, H], BF16, tag="b1b", name="b1b")
            for par, acc, dst in ((0, own1, b1a), (1, peer1, b1b)):
                nc.vector.tensor_mul(
                    dst[:], acc[:],
                    par_view(dinv_t[:], par).to_broadcast([128, NGRP, H]))

            own2, peer2 = agg_step("accA", "accB", table2)
            b2a = sb.tile([128, NGRP, H], BF16, tag="b2a", name="b2a")
            b2b = sb.tile([128, NGRP, H], BF16, tag="b2b", name="b2b")
            for par, acc, dst in ((0, own2, b2a), (1, peer2, b2b)):
                nc.vector.tensor_mul(
                    dst[:], acc[:],
                    par_view(dinv_t[:], par).to_broadcast([128, NGRP, H]))

            # ---------- emb (transposed) with on-demand B transposes
            embT = sb.tile([H + 1, NLOCP], BF16, tag="xsb")
            nc.vector.memset(embT[H:H + 1, :], 1.0)
            for s0 in range(0, NLOCP, 512):
                cw = min(512, NLOCP - s0)
                ncols_here = cw // 128
                b1T = midp.tile([H, 512], BF16, tag="b1T", name="b1T")
                b2T = midp.tile([H, 512], BF16, tag="b2T", name="b2T")
                pt1 = ps.tile([H, 512], BF16, tag="ps")
                pt2 = ps.tile([H, 512], BF16, tag="ps")
                for t in range(ncols_here):
                    col = s0 // 128 + t
                    srcs = (b1a if col % 2 == 0 else b1b,
                            b2a if col % 2 == 0 else b2b)
                    g2 = col // 2
                    nc.tensor.transpose(
                        pt1[:, t * 128:(t + 1) * 128],
                        srcs[0][:, g2:g2 + 1, :].rearrange("p o h -> p (o h)"),
                        i128[:])
                    nc.tensor.transpose(
                        pt2[:, t * 128:(t + 1) * 128],
                        srcs[1][:, g2:g2 + 1, :].rearrange("p o h -> p (o h)"),
                        i128[:])
                nc.vector.tensor_copy(b1T[:, :cw], pt1[:, :cw])
                nc.vector.tensor_copy(b2T[:, :cw], pt2[:, :cw])
                pe = ps.tile([H, 512], F32, tag="ps")
                sl = slice(s0, s0 + cw)
                nc.tensor.matmul(pe[:, :cw], lhsT=k01_t[:], rhs=h2T[:, sl],
                                 start=True, stop=False)
                nc.tensor.matmul(pe[:, :cw], lhsT=kb1_t[:], rhs=b1T[:, :cw],
                                 start=False, stop=False)
                nc.tensor.matmul(pe[:, :cw], lhsT=kb2_t[:], rhs=b2T[:, :cw],
                                 start=False, stop=True)
                nc.scalar.activation(embT[0:H, sl], pe[:, :cw], Relu,
                                     bias=bemb_t[:])

            # ---------- outputs: emb node-major + logits, packed [NLOC, 66]
            outb = sb.tile([128, NCOLS, HC], F32, tag="bigA", bufs=2)
            for g0 in range(0, NCOLS, 7):
                gn = min(7, NCOLS - g0)
                po = ps.tile([128, 7 * HC], F32, tag="ps")
                for t in range(gn):
                    col = g0 + t
                    tsl = slice(col * 128, (col + 1) * 128)
                    off = t * HC
                    nc.tensor.matmul(po[:, off:off + H],
                                     lhsT=embT[0:H, tsl], rhs=i64[:],
                                     start=True, stop=True)
                    nc.tensor.matmul(po[:, off + H:off + HC],
                                     lhsT=embT[:, tsl], rhs=w4b_t[:],
                                     start=True, stop=True)
                nc.vector.tensor_copy(
                    outb[:, g0:g0 + gn, :],
                    po[:, :gn * HC].rearrange("p (c h) -> p c h", h=HC))
            nc.sync.dma_start(
                out[:].rearrange("(c p) h -> p c h", p=128), outb[:])

    nc.compile()
    return nc


# ---------------------------------------------------------------- entry

def run(inputs, trace=False, trace_kwargs=None):
    meta, in_maps, perm = _prep(inputs)
    nc = _build(meta)
    kw = {}
    if trace:
        kw["trace"] = True
        if trace_kwargs:
            kw["trace_kwargs"] = trace_kwargs
    res = run_bass_kernel_spmd(nc, in_maps, core_ids=list(range(NCORES)), **kw)
    N = meta["N"]
    logits = np.zeros((N, C_OUT), np.float32)
    emb = np.zeros((N, H), np.float32)
    NLOC = meta["NLOC"]
    for c in range(NCORES):
        o = np.asarray(res.results[c]["out"])[:NLOC]
        emb[perm[c]] = o[:, :H]
        logits[perm[c]] = o[:, H:]
    return (logits, emb), res


def kernel(**inputs):
    (logits, emb), _ = run(inputs)
    return logits, emb


# revision 13
# speedup vs baseline: 1.2105x; 1.0117x over previous
"""AdaGNN (gnn_message_passing) distributed Bass kernel for 8 TRN2 NeuronCores.

Math refactoring (exact, up to fp reassociation):
  The reference runs 3 PolyConvs, each applying the unnormalized Laplacian
  twice (6 gather+segment_sum rounds).  All 3 convs start from the same h and
  the per-feature diagonal scales (ld) commute through the adjacency A, so
  only TWO aggregations are needed:
     M1 = A @ (h * d)          (d = deg^-1/2 per node)
     M2 = A @ (M1 * d^2)
  With B1 = M1*d, B2 = M2*d every conv output is
     h_i = th0*(h*ld_i0)@cW_i^T + cb_i + c_h(i)*h + B1*alpha_i + B2*beta_i
  and emb = relu(concat_i(h_i) @ W3^T + b3) collapses to
     emb = relu(h @ K01 + B1 @ KB1 + B2 @ KB2 + b_emb)
  with K01/KB1/KB2/b_emb folded on the host from the (tiny) parameters.

Distribution: nodes are degree-sorted and snake-assigned to the 8 cores
(dst ownership).  Each core computes its h/x shard, AllGathers the x table
([8*(NLOC+1), 64] f32, one zero row per shard), then gathers per-edge rows
with dma_gather (4 int16 windows, per-window lane-balanced slot grids, pads
point at the window's zero row) and segment-sums with dma_scatter_add's
SBUF-parity CCE accumulate (same dst always in the same lane -> same DMA
engine -> no RMW race; consecutive scatter chunks are serialized by Tile).
"""

import numpy as np
import ml_dtypes

import concourse.bass as bass
import concourse.mybir as mybir
import concourse.tile as tile
import concourse.bacc as bacc
from concourse.bass_utils import run_bass_kernel_spmd
from concourse.masks import make_identity

F32 = mybir.dt.float32
BF16 = mybir.dt.bfloat16
I16 = mybir.dt.int16
BF16NP = ml_dtypes.bfloat16

NCORES = 8
H = 64
C_OUT = 2
THETAS = ((3.0, -3.0, 0.75), (0.0, 3.0, -1.5), (0.0, 0.0, 0.75))
GCH_COLS = 48                        # gather chunk columns (385 ring descs/engine)
SCH_POS = 49                         # scatter chunk positions (all tokens unique)
IDXW = GCH_COLS * 128 // 16          # free dim of a wrapped gather idx tile
SIDXW = SCH_POS * 128 // 16


# ---------------------------------------------------------------- host prep

def _snake(n_items, n_bins):
    """rank -> bin, serpentine."""
    r = np.arange(n_items)
    blk, pos = r // n_bins, r % n_bins
    return np.where(blk % 2 == 0, pos, n_bins - 1 - pos)


def _prep(inputs):
    in_feat = np.asarray(inputs["in_feat"], np.float32)
    src = np.asarray(inputs["src"]).astype(np.int64)
    dst = np.asarray(inputs["dst"]).astype(np.int64)
    N, IN = in_feat.shape
    E = src.shape[0]
    assert N % NCORES == 0
    NLOC = N // NCORES
    NCOLS = -(-NLOC // 128)
    if NCOLS % 2:
        NCOLS += 1                      # NLOCP must be a multiple of 256
    if NCOLS * 128 <= NLOC:
        NCOLS += 2                      # spare tokens for the dummy dst
    NLOCP = NCOLS * 128
    assert NLOCP > NLOC
    SHARD = NLOCP                       # shard rows; [NLOC, NLOCP) are zeros
    TROWS = NCORES * SHARD
    WBASE = 2 * SHARD                   # window stride; zero rows at rel NLOC+
    assert WBASE <= 32768
    NW = -(-TROWS // WBASE)
    NGRP = NLOCP // 256
    DUMMY_TOK = NLOCP - 1
    PAD_REL = NLOC                      # a zero row of shard 2w, rel to base

    deg = np.bincount(dst, minlength=N).astype(np.int64)
    d_inv = (np.maximum(deg.astype(np.float32), 1.0) ** -0.5).astype(np.float32)

    order = np.argsort(-deg, kind="stable")      # rank -> orig node id
    core_of_rank = _snake(N, NCORES)
    local_of_rank = np.arange(N) // NCORES
    core = np.empty(N, np.int64)
    local = np.empty(N, np.int64)
    core[order] = core_of_rank
    local[order] = local_of_rank
    row = core * SHARD + local                   # table row of each orig node

    perm = np.empty((NCORES, NLOC), np.int64)    # perm[c][l] = orig node
    perm[core[order], local[order]] = order

    # ---- per (core, window) slot grids
    e_core = core[dst]
    s_row = row[src]
    e_w = s_row // WBASE
    s_rel = (s_row - e_w * WBASE).astype(np.int64)
    d_tok = local[dst]

    # Per (core, window): sort nodes by window in-degree (descending);
    # window-rank r sits at grid cell (partition r%128, position r//128).
    # Position k gets D[k] slot columns (max over cores of the position's
    # largest degree); a node's edges fill its cell's columns, pads point
    # at a zero table row.  A DVE segmented reduce turns the gathered grid
    # into per-node partials (window-rank order), and one dma_scatter_add
    # per ~SCH_POS positions routes them into the canonical accumulators
    # (every token distinct within a call -> no CCE RMW hazards).
    per_cw = {}
    deg_cw = np.zeros((NCORES, NW, NLOCP), np.int64)
    for c in range(NCORES):
        m_c = e_core == c
        for w in range(NW):
            m = m_c & (e_w == w)
            dt = d_tok[m]
            sr = s_rel[m]
            cnt = np.bincount(dt, minlength=NLOCP).astype(np.int64)
            deg_cw[c, w] = cnt
            nrank = np.argsort(-cnt, kind="stable")      # wrank -> node
            per_cw[(c, w)] = (dt, sr, cnt, nrank)

    NPOS = NLOCP // 128
    D_w = np.zeros((NW, NPOS), np.int64)
    for w in range(NW):
        for c in range(NCORES):
            _, _, cnt, nrank = per_cw[(c, w)]
            dsorted = cnt[nrank]
            D_w[w] = np.maximum(D_w[w], dsorted[0::128][:NPOS])
    npos_w = [int((D_w[w] > 0).sum()) for w in range(NW)]

    # gather chunks: (window, [(pos0, npos, D)]) runs packed to <=GCH_COLS
    gchunks = []
    for w in range(NW):
        runs = []          # maximal equal-D runs
        k = 0
        while k < npos_w[w]:
            j = k
            while j < npos_w[w] and D_w[w][j] == D_w[w][k]:
                j += 1
            runs.append((k, j - k, int(D_w[w][k])))
            k = j
        cur, cw = [], 0
        for (k0, nk, D) in runs:
            while nk > 0:
                fit = min(nk, max(0, (GCH_COLS - cw) // D))
                if fit == 0:
                    gchunks.append((w, cur, cw))
                    cur, cw = [], 0
                    continue
                cur.append((k0, fit, D))
                cw += fit * D
                k0 += fit
                nk -= fit
        if cur:
            gchunks.append((w, cur, cw))
    # scatter chunks: (window, pos0, npos)
    schunks = []
    for w in range(NW):
        k = 0
        while k < npos_w[w]:
            take = min(SCH_POS, npos_w[w] - k)
            schunks.append((w, k, take))
            k += take
    NGC, NSC = len(gchunks), len(schunks)

    def wrap_rep(a):
        return np.tile(a.reshape(-1, 16).T, (8, 1)).astype(np.int16)

    widx = np.zeros((NCORES, NGC, 128, IDXW), np.int16)
    sidx = np.zeros((NCORES, NSC, 128, SIDXW), np.int16)
    for c in range(NCORES):
        for w in range(NW):
            dt, sr, cnt, nrank = per_cw[(c, w)]
            wr = np.empty(NLOCP, np.int64)
            wr[nrank] = np.arange(NLOCP)
            # column offset of each position in the window grid
            C_k = np.concatenate(([0], np.cumsum(D_w[w])[:-1]))
            cols_total = int(np.cumsum(D_w[w])[-1])
            g_arr = np.full(cols_total * 128, PAD_REL, np.int64)
            if dt.size:
                r = wr[dt]
                o = np.lexsort((np.arange(dt.size), r))
                r_s = r[o]
                starts = np.searchsorted(r_s, np.arange(NLOCP))
                j = np.arange(r_s.size) - starts[r_s]
                col = C_k[r_s // 128] + j
                g = col * 128 + (r_s % 128)
                g_arr[g] = sr[o]
            # slice by gather chunks of this window
            for k, (wk, runs, cw) in enumerate(gchunks):
                if wk != w:
                    continue
                parts = []
                for (k0, nk, D) in runs:
                    c0 = int(C_k[k0])
                    parts.append(g_arr[c0 * 128:(c0 + nk * D) * 128])
                ga = np.concatenate(parts) if parts else np.zeros(0, np.int64)
                assert ga.size == cw * 128
                widx[c, k, :, : ga.size // 16] = wrap_rep(ga)
            for k, (wk, p0, npos) in enumerate(schunks):
                if wk != w:
                    continue
                toks = nrank[p0 * 128:(p0 + npos) * 128]
                toks = np.where(toks < NLOC, toks, DUMMY_TOK)
                # stream position g = j*128 + p  <->  wrank (p0+j)*128 + p
                sa = toks.reshape(npos, 128).ravel()
                sidx[c, k, :, : sa.size // 16] = wrap_rep(sa)

    # ---- folded weights (float64 for stability)
    W1 = np.asarray(inputs["W1"], np.float64)
    b1 = np.asarray(inputs["b1"], np.float64)
    W2 = np.asarray(inputs["W2"], np.float64)
    b2 = np.asarray(inputs["b2"], np.float64)
    W3 = np.asarray(inputs["W3"], np.float64)
    b3 = np.asarray(inputs["b3"], np.float64)
    W4 = np.asarray(inputs["W4"], np.float64)
    b4 = np.asarray(inputs["b4"], np.float64)
    lds = [np.asarray(inputs[f"ld{i+1}"], np.float64) for i in range(3)]
    cWs = [np.asarray(inputs[f"cW{i+1}"], np.float64) for i in range(3)]
    cbs = [np.asarray(inputs[f"cb{i+1}"], np.float64) for i in range(3)]

    K01 = np.zeros((H, H))
    KB1 = np.zeros((H, H))
    KB2 = np.zeros((H, H))
    b_emb = b3.copy()
    for i in range(3):
        th = THETAS[i]
        W3Ti = W3[:, i * H:(i + 1) * H].T          # [k, j]
        K01 += (th[1] + th[2]) * W3Ti
        K01 += th[0] * ((lds[i][0][:, None] * cWs[i].T) @ W3Ti)
        alpha = -th[1] * lds[i][1] - th[2] * (lds[i][1] + lds[i][2])
        beta = th[2] * lds[i][1] * lds[i][2]
        KB1 += alpha[:, None] * W3Ti
        KB2 += beta[:, None] * W3Ti
        b_emb += W3[:, i * H:(i + 1) * H] @ cbs[i]
    w4b = np.concatenate([W4.T, b4[None, :]], axis=0)        # [H+1, C]

    meta = dict(N=N, E=E, IN=IN, NLOC=NLOC, NCOLS=NCOLS, NLOCP=NLOCP,
                SHARD=SHARD, TROWS=TROWS, WBASE=WBASE, NW=NW, NGRP=NGRP,
                NGC=NGC, NSC=NSC, gchunks=gchunks, schunks=schunks,
                NPOS=NPOS)

    # ---- per-core input maps
    in_maps = []
    for c in range(NCORES):
        xin = np.zeros((IN, NLOCP), BF16NP)
        xin[:, :NLOC] = in_feat[perm[c]].T.astype(BF16NP)
        dinv = np.zeros((128, NCOLS, 1), np.float32)
        lidx = np.arange(NLOC)
        dinv[lidx % 128, lidx // 128, 0] = d_inv[perm[c]]
        in_maps.append({
            "xin": xin,
            "widx": widx[c].reshape(NGC * 128, IDXW),
            "sidx": sidx[c].reshape(NSC * 128, SIDXW),
            "dinv": dinv,
            "w1t": W1.T.astype(BF16NP).copy(),
            "b1": b1.astype(np.float32).reshape(H, 1),
            "w2t": W2.T.astype(BF16NP).copy(),
            "b2": b2.astype(np.float32).reshape(H, 1),
            "k01": K01.astype(BF16NP),
            "kb1": KB1.astype(BF16NP),
            "kb2": KB2.astype(BF16NP),
            "bemb": b_emb.astype(np.float32).reshape(H, 1),
            "w4b": w4b.astype(BF16NP),
        })
    return meta, in_maps, perm


# ---------------------------------------------------------------- builder

def _build(meta):
    IN = meta["IN"]
    NLOC, NCOLS, NLOCP = meta["NLOC"], meta["NCOLS"], meta["NLOCP"]
    SHARD, TROWS, WBASE = meta["SHARD"], meta["TROWS"], meta["WBASE"]
    NGRP, NGC, NSC = meta["NGRP"], meta["NGC"], meta["NSC"]
    gchunks, schunks = meta["gchunks"], meta["schunks"]
    HC = H + C_OUT
    Relu = mybir.ActivationFunctionType.Relu

    nc = bacc.Bacc("TRN2", target_bir_lowering=False, debug=False,
                   num_devices=NCORES, num_swdge_queues=2)

    xin = nc.dram_tensor("xin", [IN, NLOCP], BF16, kind="ExternalInput")
    widx = nc.dram_tensor("widx", [NGC * 128, IDXW], I16, kind="ExternalInput")
    sidx = nc.dram_tensor("sidx", [NSC * 128, SIDXW], I16, kind="ExternalInput")
    dinv = nc.dram_tensor("dinv", [128, NCOLS, 1], F32, kind="ExternalInput")
    w1t = nc.dram_tensor("w1t", [IN, H], BF16, kind="ExternalInput")
    b1 = nc.dram_tensor("b1", [H, 1], F32, kind="ExternalInput")
    w2t = nc.dram_tensor("w2t", [H, H], BF16, kind="ExternalInput")
    b2 = nc.dram_tensor("b2", [H, 1], F32, kind="ExternalInput")
    k01 = nc.dram_tensor("k01", [H, H], BF16, kind="ExternalInput")
    kb1 = nc.dram_tensor("kb1", [H, H], BF16, kind="ExternalInput")
    kb2 = nc.dram_tensor("kb2", [H, H], BF16, kind="ExternalInput")
    bemb = nc.dram_tensor("bemb", [H, 1], F32, kind="ExternalInput")
    w4b = nc.dram_tensor("w4b", [H + 1, C_OUT], BF16, kind="ExternalInput")
    out = nc.dram_tensor("out", [NLOCP, HC], F32, kind="ExternalOutput")


    with tile.TileContext(nc) as tc:
        with tc.tile_pool(name="dram", bufs=1, space="DRAM") as dram, \
             tc.tile_pool(name="cst", bufs=1) as cst, \
             tc.tile_pool(name="sb", bufs=1) as sb, \
             tc.tile_pool(name="prtp", bufs=2) as prtp, \
             tc.tile_pool(name="idxp", bufs=2) as idxp, \
             tc.tile_pool(name="midp", bufs=2) as midp, \
             tc.tile_pool(name="ps", bufs=6, space="PSUM") as ps:

            table1 = dram.tile([TROWS, H], F32, addr_space="Shared")
            table2 = dram.tile([TROWS, H], F32, addr_space="Shared")
            x1s = dram.tile([SHARD, H], F32)
            x2s = dram.tile([SHARD, H], F32)

            # ---------- constants
            w1t_t = cst.tile([IN, H], BF16)
            nc.sync.dma_start(w1t_t[:], w1t[:])
            w2t_t = cst.tile([H, H], BF16)
            nc.sync.dma_start(w2t_t[:], w2t[:])
            b1_t = cst.tile([H, 1], F32)
            nc.sync.dma_start(b1_t[:], b1[:])
            b2_t = cst.tile([H, 1], F32)
            nc.sync.dma_start(b2_t[:], b2[:])
            k01_t = cst.tile([H, H], BF16)
            nc.sync.dma_start(k01_t[:], k01[:])
            kb1_t = cst.tile([H, H], BF16)
            nc.sync.dma_start(kb1_t[:], kb1[:])
            kb2_t = cst.tile([H, H], BF16)
            nc.sync.dma_start(kb2_t[:], kb2[:])
            bemb_t = cst.tile([H, 1], F32)
            nc.sync.dma_start(bemb_t[:], bemb[:])
            w4b_t = cst.tile([H + 1, C_OUT], BF16)
            nc.sync.dma_start(w4b_t[:], w4b[:])
            dinv_t = cst.tile([128, NCOLS, 1], F32)
            nc.sync.dma_start(dinv_t[:], dinv[:])
            d2_t = cst.tile([128, NCOLS, 1], F32)
            nc.vector.tensor_mul(d2_t[:], dinv_t[:], dinv_t[:])
            i64 = cst.tile([H, H], BF16)
            make_identity(nc, i64[:])
            i128 = cst.tile([128, 128], BF16)
            make_identity(nc, i128[:])

            # ---------- MLP (transposed chain)
            xin_t = sb.tile([IN, NLOCP], BF16, tag="bigA", bufs=2)
            nc.sync.dma_start(xin_t[:], xin[:])
            h2T = sb.tile([H, NLOCP], BF16, tag="h2T")
            for s0 in range(0, NLOCP, 512):
                cw = min(512, NLOCP - s0)
                sl = slice(s0, s0 + cw)
                p1 = ps.tile([H, 512], F32, tag="ps")
                nc.tensor.matmul(p1[:, :cw], lhsT=w1t_t[:], rhs=xin_t[:, sl],
                                 start=True, stop=True)
                ht = midp.tile([H, 512], BF16, tag="ht")
                nc.scalar.activation(ht[:, :cw], p1[:, :cw], Relu, bias=b1_t[:])
                p2 = ps.tile([H, 512], F32, tag="ps")
                nc.tensor.matmul(p2[:, :cw], lhsT=w2t_t[:], rhs=ht[:, :cw],
                                 start=True, stop=True)
                nc.scalar.activation(h2T[:, sl], p2[:, :cw], Relu, bias=b2_t[:])

            # ---------- x1 = h * d (node-major), staged then DMA'd out
            x1_sb = sb.tile([128, NCOLS, H], F32, tag="xsb")
            for g0 in range(0, NCOLS, 7):
                gn = min(7, NCOLS - g0)
                px = ps.tile([128, 7 * H], F32, tag="ps")
                for t in range(gn):
                    col = g0 + t
                    nc.tensor.matmul(px[:, t * H:(t + 1) * H],
                                     lhsT=h2T[:, col * 128:(col + 1) * 128],
                                     rhs=i64[:], start=True, stop=True)
                nc.vector.tensor_mul(
                    x1_sb[:, g0:g0 + gn, :],
                    px[:, :gn * H].rearrange("p (c h) -> p c h", h=H),
                    dinv_t[:, g0:g0 + gn, :].to_broadcast([128, gn, H]))
            nc.sync.dma_start(
                x1s[:].rearrange("(c p) h -> p c h", p=128), x1_sb[:])
            nc.gpsimd.collective_compute(
                "AllGather", mybir.AluOpType.bypass,
                replica_groups=[list(range(NCORES))],
                ins=[x1s[:].opt()], outs=[table1[:].opt()])

            # ---------- aggregation steps
            def agg_step(own_tag, peer_tag, table):
                own = sb.tile([128, NGRP, H], F32, tag=own_tag, name=own_tag)
                peer = sb.tile([128, NGRP, H], F32, tag=peer_tag, name=peer_tag)
                nc.vector.memset(own[:], 0.0)
                nc.vector.memset(peer[:], 0.0)
                # per-window partial sums in window-rank order
                parts = {}
                gi_by_w = {}
                for k, (w, runs, cw) in enumerate(gchunks):
                    size = cw * 128
                    gi = idxp.tile([128, IDXW], I16, tag="gi", name="gi")
                    nc.sync.dma_start(gi[:], widx[k * 128:(k + 1) * 128, :])
                    msg = sb.tile([128, GCH_COLS, H], F32, tag="bigA",
                                    name="msg", bufs=2)
                    nc.gpsimd.dma_gather(
                        out_ap=msg[:, :cw, :],
                        in_ap=table[w * WBASE:, :],
                        idxs_ap=gi[:, : size // 16],
                        num_idxs=size, num_idxs_reg=size,
                        elem_size=H, queue_num=1,
                        single_packet=False)
                    # segmented reduce: each equal-D run in one DVE op
                    c_off = 0
                    for (k0, nk, D) in runs:
                        sc_idx = k0 // SCH_POS
                        part = parts.get((w, sc_idx))
                        if part is None:
                            part = prtp.tile([128, SCH_POS, H], F32,
                                             tag="part", name="part")
                            parts[(w, sc_idx)] = part
                        # run may straddle a scatter-chunk boundary
                        kk0, nnk = k0, nk
                        while nnk > 0:
                            sci = kk0 // SCH_POS
                            pp = parts.get((w, sci))
                            if pp is None:
                                pp = prtp.tile([128, SCH_POS, H], F32,
                                               tag="part", name="part")
                                parts[(w, sci)] = pp
                            take = min(nnk, (sci + 1) * SCH_POS - kk0)
                            src = msg[:, c_off:c_off + take * D, :].rearrange(
                                "p (k d) h -> p k h d", d=D)
                            nc.vector.tensor_reduce(
                                pp[:, kk0 - sci * SCH_POS:
                                   kk0 - sci * SCH_POS + take, :],
                                src, axis=mybir.AxisListType.X,
                                op=mybir.AluOpType.add)
                            c_off += take * D
                            kk0 += take
                            nnk -= take
                for k, (w, p0, npos) in enumerate(schunks):
                    size = npos * 128
                    si = idxp.tile([128, SIDXW], I16, tag="si", name="si")
                    nc.sync.dma_start(si[:], sidx[k * 128:(k + 1) * 128, :])
                    part = parts[(w, p0 // SCH_POS)]
                    nc.gpsimd.dma_scatter_add(
                        out_ap=own[:],
                        in_ap=part[:, :npos, :],
                        idxs_ap=si[:, : size // 16],
                        num_idxs=size, num_idxs_reg=size,
                        elem_size=H,
                        sbuf_tokens_per_rank=128,
                        parity_reg=0,
                        out_ap_other=peer[:],
                        queue_num=0, single_packet=False)
                return own, peer

            def par_view(ap3, par):
                # [128, NCOLS, X] -> the even/odd columns [128, NGRP, X]
                v = ap3.rearrange("p (c t) x -> p c t x", t=2)
                return v[:, :, par:par + 1, :].rearrange("p c o x -> p c (o x)")

            own1, peer1 = agg_step("accA", "accB", table1)

            # x2 = M1 * d^2  (M1 even cols in own1, odd cols in peer1)
            x2_sb = sb.tile([128, NCOLS, H], F32, tag="xsb")
            for par, acc in ((0, own1), (1, peer1)):
                nc.vector.tensor_mul(
                    par_view(x2_sb[:], par), acc[:],
                    par_view(d2_t[:], par).to_broadcast([128, NGRP, H]))
            nc.sync.dma_start(
                x2s[:].rearrange("(c p) h -> p c h", p=128), x2_sb[:])
            nc.gpsimd.collective_compute(
                "AllGather", mybir.AluOpType.bypass,
                replica_groups=[list(range(NCORES))],
                ins=[x2s[:].opt()], outs=[table2[:].opt()])

            # B1 = M1 * d (node-major bf16; transposed on demand at mm4)
            b1a = sb.tile([128, NGRP, H], BF16, tag="b1a", name="b1a")
            b1b = sb.tile([128, NGRP, H], BF16, tag="b1b", name="b1b")
            for par, acc, dst in ((0, own1, b1a), (1, peer1, b1b)):
                nc.vector.tensor_mul(
                    dst[:], acc[:],
                    par_view(dinv_t[:], par).to_broadcast([128, NGRP, H]))

            own2, peer2 = agg_step("accA", "accB", table2)
            b2a = sb.tile([128, NGRP, H], BF16, tag="b2a", name="b2a")
            b2b = sb.tile([128, NGRP, H], BF16, tag="b2b", name="b2b")
            for par, acc, dst in ((0, own2, b2a), (1, peer2, b2b)):
                nc.vector.tensor_mul(
                    dst[:], acc[:],
                    par_view(dinv_t[:], par).to_broadcast([128, NGRP, H]))

            # ---------- emb (transposed) with on-demand B transposes
            embT = sb.tile([H + 1, NLOCP], BF16, tag="xsb")
            nc.vector.memset(embT[H:H + 1, :], 1.0)
            for s0 in range(0, NLOCP, 512):
                cw = min(512, NLOCP - s0)
                ncols_here = cw // 128
                b1T = midp.tile([H, 512], BF16, tag="b1T", name="b1T")
                b2T = midp.tile([H, 512], BF16, tag="b2T", name="b2T")
                pt1 = ps.tile([H, 512], BF16, tag="ps")
                pt2 = ps.tile([H, 512], BF16, tag="ps")
                for t in range(ncols_here):
                    col = s0 // 128 + t
                    srcs = (b1a if col % 2 == 0 else b1b,
                            b2a if col % 2 == 0 else b2b)
                    g2 = col // 2
                    nc.tensor.transpose(
                        pt1[:, t * 128:(t + 1) * 128],
                        srcs[0][:, g2:g2 + 1, :].rearrange("p o h -> p (o h)"),
                        i128[:])
                    nc.tensor.transpose(
                        pt2[:, t * 128:(t + 1) * 128],
                        srcs[1][:, g2:g2 + 1, :].rearrange("p o h -> p (o h)"),
                        i128[:])
                nc.vector.tensor_copy(b1T[:, :cw], pt1[:, :cw])
                nc.vector.tensor_copy(b2T[:, :cw], pt2[:, :cw])
                pe = ps.tile([H, 512], F32, tag="ps")
                sl = slice(s0, s0 + cw)
                nc.tensor.matmul(pe[:, :cw], lhsT=k01_t[:], rhs=h2T[:, sl],
                                 start=True, stop=False)
                nc.tensor.matmul(pe[:, :cw], lhsT=kb1_t[:], rhs=b1T[:, :cw],
                                 start=False, stop=False)
                nc.tensor.matmul(pe[:, :cw], lhsT=kb2_t[:], rhs=b2T[:, :cw],
                                 start=False, stop=True)
                nc.scalar.activation(embT[0:H, sl], pe[:, :cw], Relu,
                                     bias=bemb_t[:])

            # ---------- outputs: emb node-major + logits, packed [NLOC, 66]
            outb = sb.tile([128, NCOLS, HC], F32, tag="bigA", bufs=2)
            for g0 in range(0, NCOLS, 7):
                gn = min(7, NCOLS - g0)
                po = ps.tile([128, 7 * HC], F32, tag="ps")
                for t in range(gn):
                    col = g0 + t
                    tsl = slice(col * 128, (col + 1) * 128)
                    off = t * HC
                    nc.tensor.matmul(po[:, off:off + H],
                                     lhsT=embT[0:H, tsl], rhs=i64[:],
                                     start=True, stop=True)
                    nc.tensor.matmul(po[:, off + H:off + HC],
                                     lhsT=embT[:, tsl], rhs=w4b_t[:],
                                     start=True, stop=True)
                nc.vector.tensor_copy(
                    outb[:, g0:g0 + gn, :],
                    po[:, :gn * HC].rearrange("p (c h) -> p c h", h=HC))
            nc.sync.dma_start(
                out[:].rearrange("(c p) h -> p c h", p=128), outb[:])

    nc.compile()
    return nc


# ---------------------------------------------------------------- entry

def run(inputs, trace=False, trace_kwargs=None):
    meta, in_maps, perm = _prep(inputs)
    nc = _build(meta)
    kw = {}
    if trace:
        kw["trace"] = True
        if trace_kwargs:
            kw["trace_kwargs"] = trace_kwargs
    res = run_bass_kernel_spmd(nc, in_maps, core_ids=list(range(NCORES)), **kw)
    N = meta["N"]
    logits = np.zeros((N, C_OUT), np.float32)
    emb = np.zeros((N, H), np.float32)
    NLOC = meta["NLOC"]
    for c in range(NCORES):
        o = np.asarray(res.results[c]["out"])[:NLOC]
        emb[perm[c]] = o[:, :H]
        logits[perm[c]] = o[:, H:]
    return (logits, emb), res


def kernel(**inputs):
    (logits, emb), _ = run(inputs)
    return logits, emb


# revision 26
# speedup vs baseline: 1.7310x; 1.4300x over previous
"""AdaGNN (gnn_message_passing) distributed Bass kernel for 8 TRN2 NeuronCores.

Math refactoring (exact, up to fp reassociation):
  The reference runs 3 PolyConvs, each applying the unnormalized Laplacian
  twice (6 gather+segment_sum rounds).  All 3 convs start from the same h and
  the per-feature diagonal scales (ld) commute through the adjacency A, so
  only TWO aggregations are needed:
     M1 = A @ (h * d)          (d = deg^-1/2 per node)
     M2 = A @ (M1 * d^2)
  With B1 = M1*d, B2 = M2*d every conv output is
     h_i = th0*(h*ld_i0)@cW_i^T + cb_i + c_h(i)*h + B1*alpha_i + B2*beta_i
  and emb = relu(concat_i(h_i) @ W3^T + b3) collapses to
     emb = relu(h @ K01 + B1 @ KB1 + B2 @ KB2 + b_emb)
  with K01/KB1/KB2/b_emb folded on the host from the (tiny) parameters.

Distribution: nodes are degree-sorted and snake-assigned to the 8 cores
(dst ownership).  Each core computes its h/x shard, AllGathers the x table
([8*(NLOC+1), 64] f32, one zero row per shard), then gathers per-edge rows
with dma_gather (4 int16 windows, per-window lane-balanced slot grids, pads
point at the window's zero row) and segment-sums with dma_scatter_add's
SBUF-parity CCE accumulate (same dst always in the same lane -> same DMA
engine -> no RMW race; consecutive scatter chunks are serialized by Tile).
"""

import numpy as np
import ml_dtypes

import concourse.bass as bass
import concourse.mybir as mybir
import concourse.tile as tile
import concourse.bacc as bacc
from concourse.bass_utils import run_bass_kernel_spmd
from concourse.masks import make_identity

F32 = mybir.dt.float32
BF16 = mybir.dt.bfloat16
I16 = mybir.dt.int16
BF16NP = ml_dtypes.bfloat16

NCORES = 8
H = 64
C_OUT = 2
THETAS = ((3.0, -3.0, 0.75), (0.0, 3.0, -1.5), (0.0, 0.0, 0.75))
GCH_COLS = 32                        # gather chunk columns
SCH_POS = 49                         # scatter chunk positions (all tokens unique)
IDXW = GCH_COLS * 128 // 16          # free dim of a wrapped gather idx tile
SIDXW = SCH_POS * 128 // 16


# ---------------------------------------------------------------- host prep

def _snake(n_items, n_bins):
    """rank -> bin, serpentine."""
    r = np.arange(n_items)
    blk, pos = r // n_bins, r % n_bins
    return np.where(blk % 2 == 0, pos, n_bins - 1 - pos)


def _prep(inputs):
    in_feat = np.asarray(inputs["in_feat"], np.float32)
    src = np.asarray(inputs["src"]).astype(np.int64)
    dst = np.asarray(inputs["dst"]).astype(np.int64)
    N, IN = in_feat.shape
    E = src.shape[0]
    assert N % NCORES == 0
    NLOC = N // NCORES
    NCOLS = -(-NLOC // 128)
    if NCOLS % 2:
        NCOLS += 1                      # NLOCP must be a multiple of 256
    if NCOLS * 128 <= NLOC:
        NCOLS += 2                      # spare tokens for the dummy dst
    NLOCP = NCOLS * 128
    assert NLOCP > NLOC
    SHARD = NLOCP                       # shard rows; [NLOC, NLOCP) are zeros
    TROWS = NCORES * SHARD
    WBASE = 2 * SHARD                   # window stride; zero rows at rel NLOC+
    assert WBASE <= 32768
    NW = -(-TROWS // WBASE)
    NGRP = NLOCP // 256
    DUMMY_TOK = NLOCP - 1
    PAD_REL = NLOC                      # a zero row of shard 2w, rel to base

    deg = np.bincount(dst, minlength=N).astype(np.int64)
    d_inv = (np.maximum(deg.astype(np.float32), 1.0) ** -0.5).astype(np.float32)

    order = np.argsort(-deg, kind="stable")      # rank -> orig node id
    core_of_rank = _snake(N, NCORES)
    local_of_rank = np.arange(N) // NCORES
    core = np.empty(N, np.int64)
    local = np.empty(N, np.int64)
    core[order] = core_of_rank
    local[order] = local_of_rank
    row = core * SHARD + local                   # table row of each orig node

    perm = np.empty((NCORES, NLOC), np.int64)    # perm[c][l] = orig node
    perm[core[order], local[order]] = order

    # ---- per (core, window) slot grids
    e_core = core[dst]
    s_row = row[src]
    e_w = s_row // WBASE
    s_rel = (s_row - e_w * WBASE).astype(np.int64)
    d_tok = local[dst]

    # Per (core, window): sort nodes by window in-degree (descending);
    # window-rank r sits at grid cell (partition r%128, position r//128).
    # Position k gets D[k] slot columns (max over cores of the position's
    # largest degree); a node's edges fill its cell's columns, pads point
    # at a zero table row.  A DVE segmented reduce turns the gathered grid
    # into per-node partials (window-rank order), and one dma_scatter_add
    # per ~SCH_POS positions routes them into the canonical accumulators
    # (every token distinct within a call -> no CCE RMW hazards).
    per_cw = {}
    deg_cw = np.zeros((NCORES, NW, NLOCP), np.int64)
    for c in range(NCORES):
        m_c = e_core == c
        for w in range(NW):
            m = m_c & (e_w == w)
            dt = d_tok[m]
            sr = s_rel[m]
            cnt = np.bincount(dt, minlength=NLOCP).astype(np.int64)
            deg_cw[c, w] = cnt
            nrank = np.argsort(-cnt, kind="stable")      # wrank -> node
            per_cw[(c, w)] = (dt, sr, cnt, nrank)

    NPOS = NLOCP // 128
    D_w = np.zeros((NW, NPOS), np.int64)
    for w in range(NW):
        for c in range(NCORES):
            _, _, cnt, nrank = per_cw[(c, w)]
            dsorted = cnt[nrank]
            D_w[w] = np.maximum(D_w[w], dsorted[0::128][:NPOS])
    npos_w = [int((D_w[w] > 0).sum()) for w in range(NW)]

    # gather chunks: (window, [(pos0, npos, D)]) runs packed to <=GCH_COLS
    gchunks = []
    for w in range(NW):
        runs = []          # maximal equal-D runs
        k = 0
        while k < npos_w[w]:
            j = k
            while j < npos_w[w] and D_w[w][j] == D_w[w][k]:
                j += 1
            runs.append((k, j - k, int(D_w[w][k])))
            k = j
        cur, cw = [], 0
        for (k0, nk, D) in runs:
            while nk > 0:
                fit = min(nk, max(0, (GCH_COLS - cw) // D))
                if fit == 0:
                    gchunks.append((w, cur, cw))
                    cur, cw = [], 0
                    continue
                cur.append((k0, fit, D))
                cw += fit * D
                k0 += fit
                nk -= fit
        if cur:
            gchunks.append((w, cur, cw))
    # scatter chunks: (window, pos0, npos)
    schunks = []
    for w in range(NW):
        k = 0
        while k < npos_w[w]:
            take = min(SCH_POS, npos_w[w] - k)
            schunks.append((w, k, take))
            k += take
    NGC, NSC = len(gchunks), len(schunks)

    def wrap_rep(a):
        return np.tile(a.reshape(-1, 16).T, (8, 1)).astype(np.int16)

    widx = np.zeros((NCORES, NGC, 128, IDXW), np.int16)
    sidx = np.zeros((NCORES, NSC, 128, SIDXW), np.int16)
    for c in range(NCORES):
        for w in range(NW):
            dt, sr, cnt, nrank = per_cw[(c, w)]
            wr = np.empty(NLOCP, np.int64)
            wr[nrank] = np.arange(NLOCP)
            # column offset of each position in the window grid
            C_k = np.concatenate(([0], np.cumsum(D_w[w])[:-1]))
            cols_total = int(np.cumsum(D_w[w])[-1])
            g_arr = np.full(cols_total * 128, PAD_REL, np.int64)
            if dt.size:
                r = wr[dt]
                o = np.lexsort((np.arange(dt.size), r))
                r_s = r[o]
                starts = np.searchsorted(r_s, np.arange(NLOCP))
                j = np.arange(r_s.size) - starts[r_s]
                col = C_k[r_s // 128] + j
                g = col * 128 + (r_s % 128)
                g_arr[g] = sr[o]
            # slice by gather chunks of this window
            for k, (wk, runs, cw) in enumerate(gchunks):
                if wk != w:
                    continue
                parts = []
                for (k0, nk, D) in runs:
                    c0 = int(C_k[k0])
                    parts.append(g_arr[c0 * 128:(c0 + nk * D) * 128])
                ga = np.concatenate(parts) if parts else np.zeros(0, np.int64)
                assert ga.size == cw * 128
                widx[c, k, :, : ga.size // 16] = wrap_rep(ga)
            for k, (wk, p0, npos) in enumerate(schunks):
                if wk != w:
                    continue
                toks = nrank[p0 * 128:(p0 + npos) * 128]
                toks = np.where(toks < NLOC, toks, DUMMY_TOK)
                # stream position g = j*128 + p  <->  wrank (p0+j)*128 + p
                sa = toks.reshape(npos, 128).ravel()
                sidx[c, k, :, : sa.size // 16] = wrap_rep(sa)

    # ---- folded weights (float64 for stability)
    W1 = np.asarray(inputs["W1"], np.float64)
    b1 = np.asarray(inputs["b1"], np.float64)
    W2 = np.asarray(inputs["W2"], np.float64)
    b2 = np.asarray(inputs["b2"], np.float64)
    W3 = np.asarray(inputs["W3"], np.float64)
    b3 = np.asarray(inputs["b3"], np.float64)
    W4 = np.asarray(inputs["W4"], np.float64)
    b4 = np.asarray(inputs["b4"], np.float64)
    lds = [np.asarray(inputs[f"ld{i+1}"], np.float64) for i in range(3)]
    cWs = [np.asarray(inputs[f"cW{i+1}"], np.float64) for i in range(3)]
    cbs = [np.asarray(inputs[f"cb{i+1}"], np.float64) for i in range(3)]

    K01 = np.zeros((H, H))
    KB1 = np.zeros((H, H))
    KB2 = np.zeros((H, H))
    b_emb = b3.copy()
    for i in range(3):
        th = THETAS[i]
        W3Ti = W3[:, i * H:(i + 1) * H].T          # [k, j]
        K01 += (th[1] + th[2]) * W3Ti
        K01 += th[0] * ((lds[i][0][:, None] * cWs[i].T) @ W3Ti)
        alpha = -th[1] * lds[i][1] - th[2] * (lds[i][1] + lds[i][2])
        beta = th[2] * lds[i][1] * lds[i][2]
        KB1 += alpha[:, None] * W3Ti
        KB2 += beta[:, None] * W3Ti
        b_emb += W3[:, i * H:(i + 1) * H] @ cbs[i]
    w4b = np.concatenate([W4.T, b4[None, :]], axis=0)        # [H+1, C]

    meta = dict(N=N, E=E, IN=IN, NLOC=NLOC, NCOLS=NCOLS, NLOCP=NLOCP,
                SHARD=SHARD, TROWS=TROWS, WBASE=WBASE, NW=NW, NGRP=NGRP,
                NGC=NGC, NSC=NSC, gchunks=gchunks, schunks=schunks,
                NPOS=NPOS)

    # ---- per-core input maps
    in_maps = []
    for c in range(NCORES):
        xin = np.zeros((IN, NLOCP), BF16NP)
        xin[:, :NLOC] = in_feat[perm[c]].T.astype(BF16NP)
        dinv = np.zeros((128, NCOLS, 1), np.float32)
        lidx = np.arange(NLOC)
        dinv[lidx % 128, lidx // 128, 0] = d_inv[perm[c]]
        in_maps.append({
            "xin": xin,
            "widx": widx[c].reshape(NGC * 128, IDXW),
            "sidx": sidx[c].reshape(NSC * 128, SIDXW),
            "dinv": dinv,
            "w1t": W1.T.astype(BF16NP).copy(),
            "b1": b1.astype(np.float32).reshape(H, 1),
            "w2t": W2.T.astype(BF16NP).copy(),
            "b2": b2.astype(np.float32).reshape(H, 1),
            "k01": K01.astype(BF16NP),
            "kb1": KB1.astype(BF16NP),
            "kb2": KB2.astype(BF16NP),
            "bemb": b_emb.astype(np.float32).reshape(H, 1),
            "w4b": w4b.astype(BF16NP),
        })
    return meta, in_maps, perm


# ---------------------------------------------------------------- builder

def _build(meta):
    IN = meta["IN"]
    NLOC, NCOLS, NLOCP = meta["NLOC"], meta["NCOLS"], meta["NLOCP"]
    SHARD, TROWS, WBASE = meta["SHARD"], meta["TROWS"], meta["WBASE"]
    NGRP, NGC, NSC = meta["NGRP"], meta["NGC"], meta["NSC"]
    NW = meta["NW"]
    gchunks, schunks = meta["gchunks"], meta["schunks"]
    HC = H + C_OUT
    Relu = mybir.ActivationFunctionType.Relu

    nc = bacc.Bacc("TRN2", target_bir_lowering=False, debug=False,
                   num_devices=NCORES, num_swdge_queues=2)

    xin = nc.dram_tensor("xin", [IN, NLOCP], BF16, kind="ExternalInput")
    widx = nc.dram_tensor("widx", [NGC * 128, IDXW], I16, kind="ExternalInput")
    sidx = nc.dram_tensor("sidx", [NSC * 128, SIDXW], I16, kind="ExternalInput")
    dinv = nc.dram_tensor("dinv", [128, NCOLS, 1], F32, kind="ExternalInput")
    w1t = nc.dram_tensor("w1t", [IN, H], BF16, kind="ExternalInput")
    b1 = nc.dram_tensor("b1", [H, 1], F32, kind="ExternalInput")
    w2t = nc.dram_tensor("w2t", [H, H], BF16, kind="ExternalInput")
    b2 = nc.dram_tensor("b2", [H, 1], F32, kind="ExternalInput")
    k01 = nc.dram_tensor("k01", [H, H], BF16, kind="ExternalInput")
    kb1 = nc.dram_tensor("kb1", [H, H], BF16, kind="ExternalInput")
    kb2 = nc.dram_tensor("kb2", [H, H], BF16, kind="ExternalInput")
    bemb = nc.dram_tensor("bemb", [H, 1], F32, kind="ExternalInput")
    w4b = nc.dram_tensor("w4b", [H + 1, C_OUT], BF16, kind="ExternalInput")
    out = nc.dram_tensor("out", [NLOCP, HC], F32, kind="ExternalOutput")


    with tile.TileContext(nc) as tc:
        with tc.tile_pool(name="dram", bufs=1, space="DRAM") as dram, \
             tc.tile_pool(name="cst", bufs=1) as cst, \
             tc.tile_pool(name="sb", bufs=1) as sb, \
             tc.tile_pool(name="prtp", bufs=2) as prtp, \
             tc.tile_pool(name="msgp", bufs=4) as msgp, \
             tc.tile_pool(name="idxp", bufs=4) as idxp, \
             tc.tile_pool(name="midp", bufs=2) as midp, \
             tc.tile_pool(name="ps", bufs=4, space="PSUM") as ps:

            table1 = dram.tile([TROWS, H], F32, addr_space="Shared")
            table2 = dram.tile([TROWS, H], F32, addr_space="Shared")
            x1s = dram.tile([SHARD, H], F32)
            x2s = dram.tile([SHARD, H], F32)

            # ---------- constants
            w1t_t = cst.tile([IN, H], BF16)
            nc.sync.dma_start(w1t_t[:], w1t[:])
            w2t_t = cst.tile([H, H], BF16)
            nc.sync.dma_start(w2t_t[:], w2t[:])
            b1_t = cst.tile([H, 1], F32)
            nc.sync.dma_start(b1_t[:], b1[:])
            b2_t = cst.tile([H, 1], F32)
            nc.sync.dma_start(b2_t[:], b2[:])
            k01_t = cst.tile([H, H], BF16)
            nc.sync.dma_start(k01_t[:], k01[:])
            kb1_t = cst.tile([H, H], BF16)
            nc.sync.dma_start(kb1_t[:], kb1[:])
            kb2_t = cst.tile([H, H], BF16)
            nc.sync.dma_start(kb2_t[:], kb2[:])
            bemb_t = cst.tile([H, 1], F32)
            nc.sync.dma_start(bemb_t[:], bemb[:])
            w4b_t = cst.tile([H + 1, C_OUT], BF16)
            nc.sync.dma_start(w4b_t[:], w4b[:])
            dinv_t = cst.tile([128, NCOLS, 1], F32)
            nc.sync.dma_start(dinv_t[:], dinv[:])
            d2_t = cst.tile([128, NCOLS, 1], F32)
            nc.vector.tensor_mul(d2_t[:], dinv_t[:], dinv_t[:])
            i64 = cst.tile([H, H], BF16)
            make_identity(nc, i64[:])
            i128 = cst.tile([128, 128], BF16)
            make_identity(nc, i128[:])

            # ---------- MLP (transposed chain)
            xin_t = sb.tile([IN, NLOCP], BF16, tag="bigA")
            nc.sync.dma_start(xin_t[:], xin[:])
            h2T = sb.tile([H, NLOCP], BF16, tag="h2T")
            for s0 in range(0, NLOCP, 512):
                cw = min(512, NLOCP - s0)
                sl = slice(s0, s0 + cw)
                p1 = ps.tile([H, 512], F32, tag="ps")
                nc.tensor.matmul(p1[:, :cw], lhsT=w1t_t[:], rhs=xin_t[:, sl],
                                 start=True, stop=True)
                ht = midp.tile([H, 512], BF16, tag="ht")
                nc.scalar.activation(ht[:, :cw], p1[:, :cw], Relu, bias=b1_t[:])
                p2 = ps.tile([H, 512], F32, tag="ps")
                nc.tensor.matmul(p2[:, :cw], lhsT=w2t_t[:], rhs=ht[:, :cw],
                                 start=True, stop=True)
                nc.scalar.activation(h2T[:, sl], p2[:, :cw], Relu, bias=b2_t[:])

            # ---------- x1 = h * d (node-major), staged then DMA'd out
            x1_sb = sb.tile([128, NCOLS, H], F32, tag="xsb")
            for g0 in range(0, NCOLS, 7):
                gn = min(7, NCOLS - g0)
                px = ps.tile([128, 7 * H], F32, tag="ps")
                for t in range(gn):
                    col = g0 + t
                    nc.tensor.matmul(px[:, t * H:(t + 1) * H],
                                     lhsT=h2T[:, col * 128:(col + 1) * 128],
                                     rhs=i64[:], start=True, stop=True)
                nc.vector.tensor_mul(
                    x1_sb[:, g0:g0 + gn, :],
                    px[:, :gn * H].rearrange("p (c h) -> p c h", h=H),
                    dinv_t[:, g0:g0 + gn, :].to_broadcast([128, gn, H]))
            nc.sync.dma_start(
                x1s[:].rearrange("(c p) h -> p c h", p=128), x1_sb[:])
            nc.gpsimd.collective_compute(
                "AllGather", mybir.AluOpType.bypass,
                replica_groups=[list(range(NCORES))],
                ins=[x1s[:].opt()], outs=[table1[:].opt()])

            # ---------- aggregation steps
            qctr = [0]

            def next_q():
                qctr[0] += 1
                return qctr[0] % 2

            def agg_step(own_tag, peer_tag, table):
                own = sb.tile([128, NGRP, H], F32, tag=own_tag, name=own_tag)
                peer = sb.tile([128, NGRP, H], F32, tag=peer_tag, name=peer_tag)
                nc.vector.memset(own[:], 0.0)
                nc.vector.memset(peer[:], 0.0)
                parts = {}

                def do_gather(k, w, runs, cw):
                    size = cw * 128
                    gi = idxp.tile([128, IDXW], I16, tag="gi", name="gi")
                    nc.sync.dma_start(gi[:], widx[k * 128:(k + 1) * 128, :])
                    msg = msgp.tile([128, GCH_COLS, H], F32, tag="msg",
                                    name="msg")
                    nc.gpsimd.dma_gather(
                        out_ap=msg[:, :cw, :],
                        in_ap=table[w * WBASE:, :],
                        idxs_ap=gi[:, : size // 16],
                        num_idxs=size, num_idxs_reg=size,
                        elem_size=H, queue_num=next_q(),
                        single_packet=False)
                    c_off = 0
                    for (k0, nk, D) in runs:
                        kk0, nnk = k0, nk
                        while nnk > 0:
                            sci = kk0 // SCH_POS
                            pp = parts.get((w, sci))
                            if pp is None:
                                pp = prtp.tile([128, SCH_POS, H], F32,
                                               tag="part", name="part")
                                parts[(w, sci)] = pp
                            take = min(nnk, (sci + 1) * SCH_POS - kk0)
                            src = msg[:, c_off:c_off + take * D, :].rearrange(
                                "p (k d) h -> p k h d", d=D)
                            nc.vector.tensor_reduce(
                                pp[:, kk0 - sci * SCH_POS:
                                   kk0 - sci * SCH_POS + take, :],
                                src, axis=mybir.AxisListType.X,
                                op=mybir.AluOpType.add)
                            c_off += take * D
                            kk0 += take
                            nnk -= take

                def do_scatter(k, w, p0, npos):
                    size = npos * 128
                    si = idxp.tile([128, SIDXW], I16, tag="si", name="si")
                    nc.sync.dma_start(si[:], sidx[k * 128:(k + 1) * 128, :])
                    part = parts[(w, p0 // SCH_POS)]
                    nc.gpsimd.dma_scatter_add(
                        out_ap=own[:],
                        in_ap=part[:, :npos, :],
                        idxs_ap=si[:, : size // 16],
                        num_idxs=size, num_idxs_reg=size,
                        elem_size=H,
                        sbuf_tokens_per_rank=128,
                        parity_reg=0,
                        out_ap_other=peer[:],
                        queue_num=next_q(), single_packet=False)

                for w in range(NW):
                    wg = [(k, runs, cw) for k, (wk, runs, cw)
                          in enumerate(gchunks) if wk == w]
                    ws = [(k, p0, npos) for k, (wk, p0, npos)
                          in enumerate(schunks) if wk == w]
                    # issue each scatter once the gather chunks covering its
                    # positions have been issued (reduces trail them closely)
                    pos_done = 0
                    si = 0
                    for (k, runs, cw) in wg:
                        do_gather(k, w, runs, cw)
                        pos_done = max(pos_done,
                                       max(k0 + nk for k0, nk, _ in runs))
                        while si < len(ws) and \
                                ws[si][1] + ws[si][2] <= pos_done - 16:
                            do_scatter(ws[si][0], w, ws[si][1], ws[si][2])
                            si += 1
                    for (k, p0, npos) in ws[si:]:
                        do_scatter(k, w, p0, npos)
                return own, peer

            def par_view(ap3, par):
                # [128, NCOLS, X] -> the even/odd columns [128, NGRP, X]
                v = ap3.rearrange("p (c t) x -> p c t x", t=2)
                return v[:, :, par:par + 1, :].rearrange("p c o x -> p c (o x)")

            own1, peer1 = agg_step("accA", "accB", table1)

            # x2 = M1 * d^2  (M1 even cols in own1, odd cols in peer1)
            x2_sb = sb.tile([128, NCOLS, H], F32, tag="xsb")
            for par, acc in ((0, own1), (1, peer1)):
                nc.vector.tensor_mul(
                    par_view(x2_sb[:], par), acc[:],
                    par_view(d2_t[:], par).to_broadcast([128, NGRP, H]))
            nc.sync.dma_start(
                x2s[:].rearrange("(c p) h -> p c h", p=128), x2_sb[:])
            nc.gpsimd.collective_compute(
                "AllGather", mybir.AluOpType.bypass,
                replica_groups=[list(range(NCORES))],
                ins=[x2s[:].opt()], outs=[table2[:].opt()])

            # B1 = M1 * d (node-major bf16; transposed on demand at mm4)
            b1a = sb.tile([128, NGRP, H], BF16, tag="b1a", name="b1a")
            b1b = sb.tile([128, NGRP, H], BF16, tag="b1b", name="b1b")
            for par, acc, dst in ((0, own1, b1a), (1, peer1, b1b)):
                nc.vector.tensor_mul(
                    dst[:], acc[:],
                    par_view(dinv_t[:], par).to_broadcast([128, NGRP, H]))

            own2, peer2 = agg_step("accA", "accB", table2)
            b2a = sb.tile([128, NGRP, H], BF16, tag="b2a", name="b2a")
            b2b = sb.tile([128, NGRP, H], BF16, tag="b2b", name="b2b")
            for par, acc, dst in ((0, own2, b2a), (1, peer2, b2b)):
                nc.vector.tensor_mul(
                    dst[:], acc[:],
                    par_view(dinv_t[:], par).to_broadcast([128, NGRP, H]))

            # ---------- emb (transposed) with on-demand B transposes
            embT = sb.tile([H + 1, NLOCP], BF16, tag="xsb")
            nc.vector.memset(embT[H:H + 1, :], 1.0)
            for s0 in range(0, NLOCP, 512):
                cw = min(512, NLOCP - s0)
                ncols_here = cw // 128
                b1T = midp.tile([H, 512], BF16, tag="b1T", name="b1T")
                b2T = midp.tile([H, 512], BF16, tag="b2T", name="b2T")
                pt1 = ps.tile([H, 512], BF16, tag="pt", bufs=4)
                pt2 = ps.tile([H, 512], BF16, tag="pt", bufs=4)
                for t in range(ncols_here):
                    col = s0 // 128 + t
                    srcs = (b1a if col % 2 == 0 else b1b,
                            b2a if col % 2 == 0 else b2b)
                    g2 = col // 2
                    nc.tensor.transpose(
                        pt1[:, t * 128:(t + 1) * 128],
                        srcs[0][:, g2:g2 + 1, :].rearrange("p o h -> p (o h)"),
                        i128[:])
                    nc.tensor.transpose(
                        pt2[:, t * 128:(t + 1) * 128],
                        srcs[1][:, g2:g2 + 1, :].rearrange("p o h -> p (o h)"),
                        i128[:])
                nc.vector.tensor_copy(b1T[:, :cw], pt1[:, :cw])
                nc.vector.tensor_copy(b2T[:, :cw], pt2[:, :cw])
                pe = ps.tile([H, 512], F32, tag="ps")
                sl = slice(s0, s0 + cw)
                nc.tensor.matmul(pe[:, :cw], lhsT=k01_t[:], rhs=h2T[:, sl],
                                 start=True, stop=False)
                nc.tensor.matmul(pe[:, :cw], lhsT=kb1_t[:], rhs=b1T[:, :cw],
                                 start=False, stop=False)
                nc.tensor.matmul(pe[:, :cw], lhsT=kb2_t[:], rhs=b2T[:, :cw],
                                 start=False, stop=True)
                nc.scalar.activation(embT[0:H, sl], pe[:, :cw], Relu,
                                     bias=bemb_t[:])

            # ---------- outputs: emb node-major + logits, packed [NLOC, 66]
            outb = sb.tile([128, NCOLS, HC], F32, tag="bigA")
            for g0 in range(0, NCOLS, 7):
                gn = min(7, NCOLS - g0)
                po = ps.tile([128, 7 * HC], F32, tag="ps")
                for t in range(gn):
                    col = g0 + t
                    tsl = slice(col * 128, (col + 1) * 128)
                    off = t * HC
                    nc.tensor.matmul(po[:, off:off + H],
                                     lhsT=embT[0:H, tsl], rhs=i64[:],
                                     start=True, stop=True)
                    nc.tensor.matmul(po[:, off + H:off + HC],
                                     lhsT=embT[:, tsl], rhs=w4b_t[:],
                                     start=True, stop=True)
                nc.vector.tensor_copy(
                    outb[:, g0:g0 + gn, :],
                    po[:, :gn * HC].rearrange("p (c h) -> p c h", h=HC))
            nc.sync.dma_start(
                out[:].rearrange("(c p) h -> p c h", p=128), outb[:])

    nc.compile()
    return nc


# ---------------------------------------------------------------- entry

def run(inputs, trace=False, trace_kwargs=None):
    meta, in_maps, perm = _prep(inputs)
    nc = _build(meta)
    kw = {}
    if trace:
        kw["trace"] = True
        if trace_kwargs:
            kw["trace_kwargs"] = trace_kwargs
    res = run_bass_kernel_spmd(nc, in_maps, core_ids=list(range(NCORES)), **kw)
    N = meta["N"]
    logits = np.zeros((N, C_OUT), np.float32)
    emb = np.zeros((N, H), np.float32)
    NLOC = meta["NLOC"]
    for c in range(NCORES):
        o = np.asarray(res.results[c]["out"])[:NLOC]
        emb[perm[c]] = o[:, :H]
        logits[perm[c]] = o[:, H:]
    return (logits, emb), res


def kernel(**inputs):
    (logits, emb), _ = run(inputs)
    return logits, emb


# revision 27
# speedup vs baseline: 1.9752x; 1.1411x over previous
"""AdaGNN (gnn_message_passing) distributed Bass kernel for 8 TRN2 NeuronCores.

Math refactoring (exact, up to fp reassociation):
  The reference runs 3 PolyConvs, each applying the unnormalized Laplacian
  twice (6 gather+segment_sum rounds).  All 3 convs start from the same h and
  the per-feature diagonal scales (ld) commute through the adjacency A, so
  only TWO aggregations are needed:
     M1 = A @ (h * d)          (d = deg^-1/2 per node)
     M2 = A @ (M1 * d^2)
  With B1 = M1*d, B2 = M2*d every conv output is
     h_i = th0*(h*ld_i0)@cW_i^T + cb_i + c_h(i)*h + B1*alpha_i + B2*beta_i
  and emb = relu(concat_i(h_i) @ W3^T + b3) collapses to
     emb = relu(h @ K01 + B1 @ KB1 + B2 @ KB2 + b_emb)
  with K01/KB1/KB2/b_emb folded on the host from the (tiny) parameters.

Distribution: nodes are degree-sorted and snake-assigned to the 8 cores
(dst ownership).  Each core computes its h/x shard, AllGathers the x table
([8*(NLOC+1), 64] f32, one zero row per shard), then gathers per-edge rows
with dma_gather (4 int16 windows, per-window lane-balanced slot grids, pads
point at the window's zero row) and segment-sums with dma_scatter_add's
SBUF-parity CCE accumulate (same dst always in the same lane -> same DMA
engine -> no RMW race; consecutive scatter chunks are serialized by Tile).
"""

import numpy as np
import ml_dtypes

import concourse.bass as bass
import concourse.mybir as mybir
import concourse.tile as tile
import concourse.bacc as bacc
from concourse.bass_utils import run_bass_kernel_spmd
from concourse.masks import make_identity

F32 = mybir.dt.float32
BF16 = mybir.dt.bfloat16
I16 = mybir.dt.int16
BF16NP = ml_dtypes.bfloat16

NCORES = 8
H = 64
C_OUT = 2
THETAS = ((3.0, -3.0, 0.75), (0.0, 3.0, -1.5), (0.0, 0.0, 0.75))
GCH_COLS = 32                        # gather chunk columns
SCH_POS = 49                         # scatter chunk positions (all tokens unique)
IDXW = GCH_COLS * 128 // 16          # free dim of a wrapped gather idx tile
SIDXW = SCH_POS * 128 // 16


# ---------------------------------------------------------------- host prep

def _snake(n_items, n_bins):
    """rank -> bin, serpentine."""
    r = np.arange(n_items)
    blk, pos = r // n_bins, r % n_bins
    return np.where(blk % 2 == 0, pos, n_bins - 1 - pos)


def _prep(inputs):
    in_feat = np.asarray(inputs["in_feat"], np.float32)
    src = np.asarray(inputs["src"]).astype(np.int64)
    dst = np.asarray(inputs["dst"]).astype(np.int64)
    N, IN = in_feat.shape
    E = src.shape[0]
    assert N % NCORES == 0
    NLOC = N // NCORES
    NCOLS = -(-NLOC // 128)
    if NCOLS % 2:
        NCOLS += 1                      # NLOCP must be a multiple of 256
    if NCOLS * 128 <= NLOC:
        NCOLS += 2                      # spare tokens for the dummy dst
    NLOCP = NCOLS * 128
    assert NLOCP > NLOC
    SHARD = NLOCP                       # shard rows; [NLOC, NLOCP) are zeros
    TROWS = NCORES * SHARD
    WBASE = 2 * SHARD                   # window stride; zero rows at rel NLOC+
    assert WBASE <= 32768
    NW = -(-TROWS // WBASE)
    NGRP = NLOCP // 256
    DUMMY_TOK = NLOCP - 1
    PAD_REL = NLOC                      # a zero row of shard 2w, rel to base

    deg = np.bincount(dst, minlength=N).astype(np.int64)
    d_inv = (np.maximum(deg.astype(np.float32), 1.0) ** -0.5).astype(np.float32)

    order = np.argsort(-deg, kind="stable")      # rank -> orig node id
    core_of_rank = _snake(N, NCORES)
    local_of_rank = np.arange(N) // NCORES
    core = np.empty(N, np.int64)
    local = np.empty(N, np.int64)
    core[order] = core_of_rank
    local[order] = local_of_rank
    row = core * SHARD + local                   # table row of each orig node

    perm = np.empty((NCORES, NLOC), np.int64)    # perm[c][l] = orig node
    perm[core[order], local[order]] = order

    # ---- per (core, window) slot grids
    e_core = core[dst]
    s_row = row[src]
    e_w = s_row // WBASE
    s_rel = (s_row - e_w * WBASE).astype(np.int64)
    d_tok = local[dst]

    # Per (core, window): sort nodes by window in-degree (descending);
    # window-rank r sits at grid cell (partition r%128, position r//128).
    # Position k gets D[k] slot columns (max over cores of the position's
    # largest degree); a node's edges fill its cell's columns, pads point
    # at a zero table row.  A DVE segmented reduce turns the gathered grid
    # into per-node partials (window-rank order), and one dma_scatter_add
    # per ~SCH_POS positions routes them into the canonical accumulators
    # (every token distinct within a call -> no CCE RMW hazards).
    per_cw = {}
    deg_cw = np.zeros((NCORES, NW, NLOCP), np.int64)
    for c in range(NCORES):
        m_c = e_core == c
        for w in range(NW):
            m = m_c & (e_w == w)
            dt = d_tok[m]
            sr = s_rel[m]
            cnt = np.bincount(dt, minlength=NLOCP).astype(np.int64)
            deg_cw[c, w] = cnt
            nrank = np.argsort(-cnt, kind="stable")      # wrank -> node
            per_cw[(c, w)] = (dt, sr, cnt, nrank)

    NPOS = NLOCP // 128
    D_w = np.zeros((NW, NPOS), np.int64)
    for w in range(NW):
        for c in range(NCORES):
            _, _, cnt, nrank = per_cw[(c, w)]
            dsorted = cnt[nrank]
            D_w[w] = np.maximum(D_w[w], dsorted[0::128][:NPOS])
    npos_w = [int((D_w[w] > 0).sum()) for w in range(NW)]

    # gather chunks: (window, [(pos0, npos, D)]) runs packed to <=GCH_COLS
    gchunks = []
    for w in range(NW):
        runs = []          # maximal equal-D runs
        k = 0
        while k < npos_w[w]:
            j = k
            while j < npos_w[w] and D_w[w][j] == D_w[w][k]:
                j += 1
            runs.append((k, j - k, int(D_w[w][k])))
            k = j
        cur, cw = [], 0
        for (k0, nk, D) in runs:
            while nk > 0:
                fit = min(nk, max(0, (GCH_COLS - cw) // D))
                if fit == 0:
                    gchunks.append((w, cur, cw))
                    cur, cw = [], 0
                    continue
                cur.append((k0, fit, D))
                cw += fit * D
                k0 += fit
                nk -= fit
        if cur:
            gchunks.append((w, cur, cw))
    # scatter chunks: (window, pos0, npos)
    schunks = []
    for w in range(NW):
        k = 0
        while k < npos_w[w]:
            take = min(SCH_POS, npos_w[w] - k)
            schunks.append((w, k, take))
            k += take
    NGC, NSC = len(gchunks), len(schunks)

    def wrap_rep(a):
        return np.tile(a.reshape(-1, 16).T, (8, 1)).astype(np.int16)

    widx = np.zeros((NCORES, NGC, 128, IDXW), np.int16)
    sidx = np.zeros((NCORES, NSC, 128, SIDXW), np.int16)
    for c in range(NCORES):
        for w in range(NW):
            dt, sr, cnt, nrank = per_cw[(c, w)]
            wr = np.empty(NLOCP, np.int64)
            wr[nrank] = np.arange(NLOCP)
            # column offset of each position in the window grid
            C_k = np.concatenate(([0], np.cumsum(D_w[w])[:-1]))
            cols_total = int(np.cumsum(D_w[w])[-1])
            g_arr = np.full(cols_total * 128, PAD_REL, np.int64)
            if dt.size:
                r = wr[dt]
                o = np.lexsort((np.arange(dt.size), r))
                r_s = r[o]
                starts = np.searchsorted(r_s, np.arange(NLOCP))
                j = np.arange(r_s.size) - starts[r_s]
                col = C_k[r_s // 128] + j
                g = col * 128 + (r_s % 128)
                g_arr[g] = sr[o]
            # slice by gather chunks of this window
            for k, (wk, runs, cw) in enumerate(gchunks):
                if wk != w:
                    continue
                parts = []
                for (k0, nk, D) in runs:
                    c0 = int(C_k[k0])
                    parts.append(g_arr[c0 * 128:(c0 + nk * D) * 128])
                ga = np.concatenate(parts) if parts else np.zeros(0, np.int64)
                assert ga.size == cw * 128
                widx[c, k, :, : ga.size // 16] = wrap_rep(ga)
            for k, (wk, p0, npos) in enumerate(schunks):
                if wk != w:
                    continue
                toks = nrank[p0 * 128:(p0 + npos) * 128]
                toks = np.where(toks < NLOC, toks, DUMMY_TOK)
                # stream position g = j*128 + p  <->  wrank (p0+j)*128 + p
                sa = toks.reshape(npos, 128).ravel()
                sidx[c, k, :, : sa.size // 16] = wrap_rep(sa)

    # ---- folded weights (float64 for stability)
    W1 = np.asarray(inputs["W1"], np.float64)
    b1 = np.asarray(inputs["b1"], np.float64)
    W2 = np.asarray(inputs["W2"], np.float64)
    b2 = np.asarray(inputs["b2"], np.float64)
    W3 = np.asarray(inputs["W3"], np.float64)
    b3 = np.asarray(inputs["b3"], np.float64)
    W4 = np.asarray(inputs["W4"], np.float64)
    b4 = np.asarray(inputs["b4"], np.float64)
    lds = [np.asarray(inputs[f"ld{i+1}"], np.float64) for i in range(3)]
    cWs = [np.asarray(inputs[f"cW{i+1}"], np.float64) for i in range(3)]
    cbs = [np.asarray(inputs[f"cb{i+1}"], np.float64) for i in range(3)]

    K01 = np.zeros((H, H))
    KB1 = np.zeros((H, H))
    KB2 = np.zeros((H, H))
    b_emb = b3.copy()
    for i in range(3):
        th = THETAS[i]
        W3Ti = W3[:, i * H:(i + 1) * H].T          # [k, j]
        K01 += (th[1] + th[2]) * W3Ti
        K01 += th[0] * ((lds[i][0][:, None] * cWs[i].T) @ W3Ti)
        alpha = -th[1] * lds[i][1] - th[2] * (lds[i][1] + lds[i][2])
        beta = th[2] * lds[i][1] * lds[i][2]
        KB1 += alpha[:, None] * W3Ti
        KB2 += beta[:, None] * W3Ti
        b_emb += W3[:, i * H:(i + 1) * H] @ cbs[i]
    w4b = np.concatenate([W4.T, b4[None, :]], axis=0)        # [H+1, C]

    meta = dict(N=N, E=E, IN=IN, NLOC=NLOC, NCOLS=NCOLS, NLOCP=NLOCP,
                SHARD=SHARD, TROWS=TROWS, WBASE=WBASE, NW=NW, NGRP=NGRP,
                NGC=NGC, NSC=NSC, gchunks=gchunks, schunks=schunks,
                NPOS=NPOS)

    # ---- per-core input maps
    in_maps = []
    for c in range(NCORES):
        xin = np.zeros((IN, NLOCP), BF16NP)
        xin[:, :NLOC] = in_feat[perm[c]].T.astype(BF16NP)
        dinv = np.zeros((128, NCOLS, 1), np.float32)
        lidx = np.arange(NLOC)
        dinv[lidx % 128, lidx // 128, 0] = d_inv[perm[c]]
        in_maps.append({
            "xin": xin,
            "widx": widx[c].reshape(NGC * 128, IDXW),
            "sidx": sidx[c].reshape(NSC * 128, SIDXW),
            "dinv": dinv,
            "w1t": W1.T.astype(BF16NP).copy(),
            "b1": b1.astype(np.float32).reshape(H, 1),
            "w2t": W2.T.astype(BF16NP).copy(),
            "b2": b2.astype(np.float32).reshape(H, 1),
            "k01": K01.astype(BF16NP),
            "kb1": KB1.astype(BF16NP),
            "kb2": KB2.astype(BF16NP),
            "bemb": b_emb.astype(np.float32).reshape(H, 1),
            "w4b": w4b.astype(BF16NP),
        })
    return meta, in_maps, perm


# ---------------------------------------------------------------- builder

def _build(meta):
    IN = meta["IN"]
    NLOC, NCOLS, NLOCP = meta["NLOC"], meta["NCOLS"], meta["NLOCP"]
    SHARD, TROWS, WBASE = meta["SHARD"], meta["TROWS"], meta["WBASE"]
    NGRP, NGC, NSC = meta["NGRP"], meta["NGC"], meta["NSC"]
    NW = meta["NW"]
    gchunks, schunks = meta["gchunks"], meta["schunks"]
    HC = H + C_OUT
    Relu = mybir.ActivationFunctionType.Relu

    nc = bacc.Bacc("TRN2", target_bir_lowering=False, debug=False,
                   num_devices=NCORES, num_swdge_queues=2)

    xin = nc.dram_tensor("xin", [IN, NLOCP], BF16, kind="ExternalInput")
    widx = nc.dram_tensor("widx", [NGC * 128, IDXW], I16, kind="ExternalInput")
    sidx = nc.dram_tensor("sidx", [NSC * 128, SIDXW], I16, kind="ExternalInput")
    dinv = nc.dram_tensor("dinv", [128, NCOLS, 1], F32, kind="ExternalInput")
    w1t = nc.dram_tensor("w1t", [IN, H], BF16, kind="ExternalInput")
    b1 = nc.dram_tensor("b1", [H, 1], F32, kind="ExternalInput")
    w2t = nc.dram_tensor("w2t", [H, H], BF16, kind="ExternalInput")
    b2 = nc.dram_tensor("b2", [H, 1], F32, kind="ExternalInput")
    k01 = nc.dram_tensor("k01", [H, H], BF16, kind="ExternalInput")
    kb1 = nc.dram_tensor("kb1", [H, H], BF16, kind="ExternalInput")
    kb2 = nc.dram_tensor("kb2", [H, H], BF16, kind="ExternalInput")
    bemb = nc.dram_tensor("bemb", [H, 1], F32, kind="ExternalInput")
    w4b = nc.dram_tensor("w4b", [H + 1, C_OUT], BF16, kind="ExternalInput")
    out = nc.dram_tensor("out", [NLOCP, HC], F32, kind="ExternalOutput")


    with tile.TileContext(nc) as tc:
        with tc.tile_pool(name="dram", bufs=1, space="DRAM") as dram, \
             tc.tile_pool(name="cst", bufs=1) as cst, \
             tc.tile_pool(name="sb", bufs=1) as sb, \
             tc.tile_pool(name="prtp", bufs=2) as prtp, \
             tc.tile_pool(name="msgp", bufs=4) as msgp, \
             tc.tile_pool(name="idxp", bufs=4) as idxp, \
             tc.tile_pool(name="midp", bufs=2) as midp, \
             tc.tile_pool(name="ps", bufs=4, space="PSUM") as ps:

            table1 = dram.tile([TROWS, H], F32, addr_space="Shared")
            table2 = dram.tile([TROWS, H], F32, addr_space="Shared")
            x1s = dram.tile([SHARD, H], F32)
            x2s = dram.tile([SHARD, H], F32)

            # ---------- constants
            w1t_t = cst.tile([IN, H], BF16)
            nc.sync.dma_start(w1t_t[:], w1t[:])
            w2t_t = cst.tile([H, H], BF16)
            nc.sync.dma_start(w2t_t[:], w2t[:])
            b1_t = cst.tile([H, 1], F32)
            nc.sync.dma_start(b1_t[:], b1[:])
            b2_t = cst.tile([H, 1], F32)
            nc.sync.dma_start(b2_t[:], b2[:])
            k01_t = cst.tile([H, H], BF16)
            nc.sync.dma_start(k01_t[:], k01[:])
            kb1_t = cst.tile([H, H], BF16)
            nc.sync.dma_start(kb1_t[:], kb1[:])
            kb2_t = cst.tile([H, H], BF16)
            nc.sync.dma_start(kb2_t[:], kb2[:])
            bemb_t = cst.tile([H, 1], F32)
            nc.sync.dma_start(bemb_t[:], bemb[:])
            w4b_t = cst.tile([H + 1, C_OUT], BF16)
            nc.sync.dma_start(w4b_t[:], w4b[:])
            dinv_t = cst.tile([128, NCOLS, 1], F32)
            nc.sync.dma_start(dinv_t[:], dinv[:])
            d2_t = cst.tile([128, NCOLS, 1], F32)
            nc.vector.tensor_mul(d2_t[:], dinv_t[:], dinv_t[:])
            i64 = cst.tile([H, H], BF16)
            make_identity(nc, i64[:])
            i128 = cst.tile([128, 128], BF16)
            make_identity(nc, i128[:])

            # ---------- MLP (transposed chain)
            xin_t = sb.tile([IN, NLOCP], BF16, tag="bigA")
            nc.sync.dma_start(xin_t[:], xin[:])
            h2T = sb.tile([H, NLOCP], BF16, tag="h2T")
            for s0 in range(0, NLOCP, 512):
                cw = min(512, NLOCP - s0)
                sl = slice(s0, s0 + cw)
                p1 = ps.tile([H, 512], F32, tag="ps")
                nc.tensor.matmul(p1[:, :cw], lhsT=w1t_t[:], rhs=xin_t[:, sl],
                                 start=True, stop=True)
                ht = midp.tile([H, 512], BF16, tag="ht")
                nc.scalar.activation(ht[:, :cw], p1[:, :cw], Relu, bias=b1_t[:])
                p2 = ps.tile([H, 512], F32, tag="ps")
                nc.tensor.matmul(p2[:, :cw], lhsT=w2t_t[:], rhs=ht[:, :cw],
                                 start=True, stop=True)
                nc.scalar.activation(h2T[:, sl], p2[:, :cw], Relu, bias=b2_t[:])

            # ---------- x1 = h * d (node-major), staged then DMA'd out
            x1_sb = sb.tile([128, NCOLS, H], F32, tag="xsb")
            for g0 in range(0, NCOLS, 7):
                gn = min(7, NCOLS - g0)
                px = ps.tile([128, 7 * H], F32, tag="ps")
                for t in range(gn):
                    col = g0 + t
                    nc.tensor.matmul(px[:, t * H:(t + 1) * H],
                                     lhsT=h2T[:, col * 128:(col + 1) * 128],
                                     rhs=i64[:], start=True, stop=True)
                nc.vector.tensor_mul(
                    x1_sb[:, g0:g0 + gn, :],
                    px[:, :gn * H].rearrange("p (c h) -> p c h", h=H),
                    dinv_t[:, g0:g0 + gn, :].to_broadcast([128, gn, H]))
            nc.sync.dma_start(
                x1s[:].rearrange("(c p) h -> p c h", p=128), x1_sb[:])
            nc.gpsimd.collective_compute(
                "AllGather", mybir.AluOpType.bypass,
                replica_groups=[list(range(NCORES))],
                ins=[x1s[:].opt()], outs=[table1[:].opt()])

            # ---------- aggregation steps
            qctr = [0]

            def next_q():
                qctr[0] += 1
                return qctr[0] % 2

            def agg_step(own_tag, peer_tag, table):
                own = sb.tile([128, NGRP, H], F32, tag=own_tag, name=own_tag)
                peer = sb.tile([128, NGRP, H], F32, tag=peer_tag, name=peer_tag)
                nc.vector.memset(own[:], 0.0)
                nc.vector.memset(peer[:], 0.0)
                parts = {}

                def do_gather(k, w, runs, cw):
                    size = cw * 128
                    gi = idxp.tile([128, IDXW], I16, tag="gi", name="gi")
                    nc.sync.dma_start(gi[:], widx[k * 128:(k + 1) * 128, :])
                    msg = msgp.tile([128, GCH_COLS, H], F32, tag="msg",
                                    name="msg")
                    nc.gpsimd.dma_gather(
                        out_ap=msg[:, :cw, :],
                        in_ap=table[w * WBASE:, :],
                        idxs_ap=gi[:, : size // 16],
                        num_idxs=size, num_idxs_reg=size,
                        elem_size=H, queue_num=next_q(),
                        single_packet=False)
                    c_off = 0
                    for (k0, nk, D) in runs:
                        kk0, nnk = k0, nk
                        while nnk > 0:
                            sci = kk0 // SCH_POS
                            pp = parts.get((w, sci))
                            if pp is None:
                                pp = prtp.tile([128, SCH_POS, H], F32,
                                               tag="part", name="part")
                                parts[(w, sci)] = pp
                            take = min(nnk, (sci + 1) * SCH_POS - kk0)
                            src = msg[:, c_off:c_off + take * D, :].rearrange(
                                "p (k d) h -> p k h d", d=D)
                            nc.vector.tensor_reduce(
                                pp[:, kk0 - sci * SCH_POS:
                                   kk0 - sci * SCH_POS + take, :],
                                src, axis=mybir.AxisListType.X,
                                op=mybir.AluOpType.add)
                            c_off += take * D
                            kk0 += take
                            nnk -= take

                def do_scatter(k, w, p0, npos):
                    size = npos * 128
                    si = idxp.tile([128, SIDXW], I16, tag="si", name="si")
                    nc.sync.dma_start(si[:], sidx[k * 128:(k + 1) * 128, :])
                    part = parts[(w, p0 // SCH_POS)]
                    nc.gpsimd.dma_scatter_add(
                        out_ap=own[:],
                        in_ap=part[:, :npos, :],
                        idxs_ap=si[:, : size // 16],
                        num_idxs=size, num_idxs_reg=size,
                        elem_size=H,
                        sbuf_tokens_per_rank=128,
                        parity_reg=0,
                        out_ap_other=peer[:],
                        queue_num=next_q(), single_packet=False)

                for w in range(NW):
                    wg = [(k, runs, cw) for k, (wk, runs, cw)
                          in enumerate(gchunks) if wk == w]
                    ws = [(k, p0, npos) for k, (wk, p0, npos)
                          in enumerate(schunks) if wk == w]
                    # issue each scatter once the gather chunks covering its
                    # positions have been issued (reduces trail them closely)
                    pos_done = 0
                    si = 0
                    for (k, runs, cw) in wg:
                        do_gather(k, w, runs, cw)
                        pos_done = max(pos_done,
                                       max(k0 + nk for k0, nk, _ in runs))
                        while si < len(ws) and \
                                ws[si][1] + ws[si][2] <= pos_done - 24:
                            do_scatter(ws[si][0], w, ws[si][1], ws[si][2])
                            si += 1
                    for (k, p0, npos) in ws[si:]:
                        do_scatter(k, w, p0, npos)
                return own, peer

            def par_view(ap3, par):
                # [128, NCOLS, X] -> the even/odd columns [128, NGRP, X]
                v = ap3.rearrange("p (c t) x -> p c t x", t=2)
                return v[:, :, par:par + 1, :].rearrange("p c o x -> p c (o x)")

            own1, peer1 = agg_step("accA", "accB", table1)

            # x2 = M1 * d^2  (M1 even cols in own1, odd cols in peer1)
            x2_sb = sb.tile([128, NCOLS, H], F32, tag="xsb")
            for par, acc in ((0, own1), (1, peer1)):
                nc.vector.tensor_mul(
                    par_view(x2_sb[:], par), acc[:],
                    par_view(d2_t[:], par).to_broadcast([128, NGRP, H]))
            nc.sync.dma_start(
                x2s[:].rearrange("(c p) h -> p c h", p=128), x2_sb[:])
            nc.gpsimd.collective_compute(
                "AllGather", mybir.AluOpType.bypass,
                replica_groups=[list(range(NCORES))],
                ins=[x2s[:].opt()], outs=[table2[:].opt()])

            # B1 = M1 * d (node-major bf16; transposed on demand at mm4)
            b1a = sb.tile([128, NGRP, H], BF16, tag="b1a", name="b1a")
            b1b = sb.tile([128, NGRP, H], BF16, tag="b1b", name="b1b")
            for par, acc, dst in ((0, own1, b1a), (1, peer1, b1b)):
                nc.vector.tensor_mul(
                    dst[:], acc[:],
                    par_view(dinv_t[:], par).to_broadcast([128, NGRP, H]))

            own2, peer2 = agg_step("accA", "accB", table2)
            b2a = sb.tile([128, NGRP, H], BF16, tag="b2a", name="b2a")
            b2b = sb.tile([128, NGRP, H], BF16, tag="b2b", name="b2b")
            for par, acc, dst in ((0, own2, b2a), (1, peer2, b2b)):
                nc.vector.tensor_mul(
                    dst[:], acc[:],
                    par_view(dinv_t[:], par).to_broadcast([128, NGRP, H]))

            # ---------- emb (transposed) with on-demand B transposes
            embT = sb.tile([H + 1, NLOCP], BF16, tag="xsb")
            nc.vector.memset(embT[H:H + 1, :], 1.0)
            for s0 in range(0, NLOCP, 512):
                cw = min(512, NLOCP - s0)
                ncols_here = cw // 128
                b1T = midp.tile([H, 512], BF16, tag="b1T", name="b1T")
                b2T = midp.tile([H, 512], BF16, tag="b2T", name="b2T")
                pt1 = ps.tile([H, 512], BF16, tag="pt", bufs=4)
                pt2 = ps.tile([H, 512], BF16, tag="pt", bufs=4)
                for t in range(ncols_here):
                    col = s0 // 128 + t
                    srcs = (b1a if col % 2 == 0 else b1b,
                            b2a if col % 2 == 0 else b2b)
                    g2 = col // 2
                    nc.tensor.transpose(
                        pt1[:, t * 128:(t + 1) * 128],
                        srcs[0][:, g2:g2 + 1, :].rearrange("p o h -> p (o h)"),
                        i128[:])
                    nc.tensor.transpose(
                        pt2[:, t * 128:(t + 1) * 128],
                        srcs[1][:, g2:g2 + 1, :].rearrange("p o h -> p (o h)"),
                        i128[:])
                nc.vector.tensor_copy(b1T[:, :cw], pt1[:, :cw])
                nc.vector.tensor_copy(b2T[:, :cw], pt2[:, :cw])
                pe = ps.tile([H, 512], F32, tag="ps")
                sl = slice(s0, s0 + cw)
                nc.tensor.matmul(pe[:, :cw], lhsT=k01_t[:], rhs=h2T[:, sl],
                                 start=True, stop=False)
                nc.tensor.matmul(pe[:, :cw], lhsT=kb1_t[:], rhs=b1T[:, :cw],
                                 start=False, stop=False)
                nc.tensor.matmul(pe[:, :cw], lhsT=kb2_t[:], rhs=b2T[:, :cw],
                                 start=False, stop=True)
                nc.scalar.activation(embT[0:H, sl], pe[:, :cw], Relu,
                                     bias=bemb_t[:])

            # ---------- outputs: emb node-major + logits, packed [NLOC, 66]
            outb = sb.tile([128, NCOLS, HC], F32, tag="bigA")
            for g0 in range(0, NCOLS, 7):
                gn = min(7, NCOLS - g0)
                po = ps.tile([128, 7 * HC], F32, tag="ps")
                for t in range(gn):
                    col = g0 + t
                    tsl = slice(col * 128, (col + 1) * 128)
                    off = t * HC
                    nc.tensor.matmul(po[:, off:off + H],
                                     lhsT=embT[0:H, tsl], rhs=i64[:],
                                     start=True, stop=True)
                    nc.tensor.matmul(po[:, off + H:off + HC],
                                     lhsT=embT[:, tsl], rhs=w4b_t[:],
                                     start=True, stop=True)
                nc.vector.tensor_copy(
                    outb[:, g0:g0 + gn, :],
                    po[:, :gn * HC].rearrange("p (c h) -> p c h", h=HC))
            nc.sync.dma_start(
                out[:].rearrange("(c p) h -> p c h", p=128), outb[:])

    nc.compile()
    return nc


# ---------------------------------------------------------------- entry

def run(inputs, trace=False, trace_kwargs=None):
    meta, in_maps, perm = _prep(inputs)
    nc = _build(meta)
    kw = {}
    if trace:
        kw["trace"] = True
        if trace_kwargs:
            kw["trace_kwargs"] = trace_kwargs
    res = run_bass_kernel_spmd(nc, in_maps, core_ids=list(range(NCORES)), **kw)
    N = meta["N"]
    logits = np.zeros((N, C_OUT), np.float32)
    emb = np.zeros((N, H), np.float32)
    NLOC = meta["NLOC"]
    for c in range(NCORES):
        o = np.asarray(res.results[c]["out"])[:NLOC]
        emb[perm[c]] = o[:, :H]
        logits[perm[c]] = o[:, H:]
    return (logits, emb), res


def kernel(**inputs):
    (logits, emb), _ = run(inputs)
    return logits, emb


# revision 28
# speedup vs baseline: 1.9957x; 1.0104x over previous
"""AdaGNN (gnn_message_passing) distributed Bass kernel for 8 TRN2 NeuronCores.

Math refactoring (exact, up to fp reassociation):
  The reference runs 3 PolyConvs, each applying the unnormalized Laplacian
  twice (6 gather+segment_sum rounds).  All 3 convs start from the same h and
  the per-feature diagonal scales (ld) commute through the adjacency A, so
  only TWO aggregations are needed:
     M1 = A @ (h * d)          (d = deg^-1/2 per node)
     M2 = A @ (M1 * d^2)
  With B1 = M1*d, B2 = M2*d every conv output is
     h_i = th0*(h*ld_i0)@cW_i^T + cb_i + c_h(i)*h + B1*alpha_i + B2*beta_i
  and emb = relu(concat_i(h_i) @ W3^T + b3) collapses to
     emb = relu(h @ K01 + B1 @ KB1 + B2 @ KB2 + b_emb)
  with K01/KB1/KB2/b_emb folded on the host from the (tiny) parameters.

Distribution: nodes are degree-sorted and snake-assigned to the 8 cores
(dst ownership).  Each core computes its h/x shard, AllGathers the x table
([8*(NLOC+1), 64] f32, one zero row per shard), then gathers per-edge rows
with dma_gather (4 int16 windows, per-window lane-balanced slot grids, pads
point at the window's zero row) and segment-sums with dma_scatter_add's
SBUF-parity CCE accumulate (same dst always in the same lane -> same DMA
engine -> no RMW race; consecutive scatter chunks are serialized by Tile).
"""

import numpy as np
import ml_dtypes

import concourse.bass as bass
import concourse.mybir as mybir
import concourse.tile as tile
import concourse.bacc as bacc
from concourse.bass_utils import run_bass_kernel_spmd
from concourse.masks import make_identity

F32 = mybir.dt.float32
BF16 = mybir.dt.bfloat16
I16 = mybir.dt.int16
BF16NP = ml_dtypes.bfloat16

NCORES = 8
H = 64
C_OUT = 2
THETAS = ((3.0, -3.0, 0.75), (0.0, 3.0, -1.5), (0.0, 0.0, 0.75))
GCH_COLS = 32                        # gather chunk columns
SCH_POS = 49                         # scatter chunk positions (all tokens unique)
IDXW = GCH_COLS * 128 // 16          # free dim of a wrapped gather idx tile
SIDXW = SCH_POS * 128 // 16


# ---------------------------------------------------------------- host prep

def _snake(n_items, n_bins):
    """rank -> bin, serpentine."""
    r = np.arange(n_items)
    blk, pos = r // n_bins, r % n_bins
    return np.where(blk % 2 == 0, pos, n_bins - 1 - pos)


def _prep(inputs):
    in_feat = np.asarray(inputs["in_feat"], np.float32)
    src = np.asarray(inputs["src"]).astype(np.int64)
    dst = np.asarray(inputs["dst"]).astype(np.int64)
    N, IN = in_feat.shape
    E = src.shape[0]
    assert N % NCORES == 0
    NLOC = N // NCORES
    NCOLS = -(-NLOC // 128)
    if NCOLS % 2:
        NCOLS += 1                      # NLOCP must be a multiple of 256
    if NCOLS * 128 <= NLOC:
        NCOLS += 2                      # spare tokens for the dummy dst
    NLOCP = NCOLS * 128
    assert NLOCP > NLOC
    SHARD = NLOCP                       # shard rows; [NLOC, NLOCP) are zeros
    TROWS = NCORES * SHARD
    WBASE = 2 * SHARD                   # window stride; zero rows at rel NLOC+
    assert WBASE <= 32768
    NW = -(-TROWS // WBASE)
    NGRP = NLOCP // 256
    DUMMY_TOK = NLOCP - 1
    PAD_REL = NLOC                      # a zero row of shard 2w, rel to base

    deg = np.bincount(dst, minlength=N).astype(np.int64)
    d_inv = (np.maximum(deg.astype(np.float32), 1.0) ** -0.5).astype(np.float32)

    order = np.argsort(-deg, kind="stable")      # rank -> orig node id
    core_of_rank = _snake(N, NCORES)
    local_of_rank = np.arange(N) // NCORES
    core = np.empty(N, np.int64)
    local = np.empty(N, np.int64)
    core[order] = core_of_rank
    local[order] = local_of_rank
    row = core * SHARD + local                   # table row of each orig node

    perm = np.empty((NCORES, NLOC), np.int64)    # perm[c][l] = orig node
    perm[core[order], local[order]] = order

    # ---- per (core, window) slot grids
    e_core = core[dst]
    s_row = row[src]
    e_w = s_row // WBASE
    s_rel = (s_row - e_w * WBASE).astype(np.int64)
    d_tok = local[dst]

    # Per (core, window): sort nodes by window in-degree (descending);
    # window-rank r sits at grid cell (partition r%128, position r//128).
    # Position k gets D[k] slot columns (max over cores of the position's
    # largest degree); a node's edges fill its cell's columns, pads point
    # at a zero table row.  A DVE segmented reduce turns the gathered grid
    # into per-node partials (window-rank order), and one dma_scatter_add
    # per ~SCH_POS positions routes them into the canonical accumulators
    # (every token distinct within a call -> no CCE RMW hazards).
    per_cw = {}
    deg_cw = np.zeros((NCORES, NW, NLOCP), np.int64)
    for c in range(NCORES):
        m_c = e_core == c
        for w in range(NW):
            m = m_c & (e_w == w)
            dt = d_tok[m]
            sr = s_rel[m]
            cnt = np.bincount(dt, minlength=NLOCP).astype(np.int64)
            deg_cw[c, w] = cnt
            nrank = np.argsort(-cnt, kind="stable")      # wrank -> node
            per_cw[(c, w)] = (dt, sr, cnt, nrank)

    NPOS = NLOCP // 128
    D_w = np.zeros((NW, NPOS), np.int64)
    for w in range(NW):
        for c in range(NCORES):
            _, _, cnt, nrank = per_cw[(c, w)]
            dsorted = cnt[nrank]
            D_w[w] = np.maximum(D_w[w], dsorted[0::128][:NPOS])
    npos_w = [int((D_w[w] > 0).sum()) for w in range(NW)]

    # gather chunks: (window, [(pos0, npos, D)]) runs packed to <=GCH_COLS
    gchunks = []
    for w in range(NW):
        runs = []          # maximal equal-D runs
        k = 0
        while k < npos_w[w]:
            j = k
            while j < npos_w[w] and D_w[w][j] == D_w[w][k]:
                j += 1
            runs.append((k, j - k, int(D_w[w][k])))
            k = j
        cur, cw = [], 0
        for (k0, nk, D) in runs:
            while nk > 0:
                fit = min(nk, max(0, (GCH_COLS - cw) // D))
                if fit == 0:
                    gchunks.append((w, cur, cw))
                    cur, cw = [], 0
                    continue
                cur.append((k0, fit, D))
                cw += fit * D
                k0 += fit
                nk -= fit
        if cur:
            gchunks.append((w, cur, cw))
    # scatter chunks: (window, pos0, npos)
    schunks = []
    for w in range(NW):
        k = 0
        while k < npos_w[w]:
            take = min(SCH_POS, npos_w[w] - k)
            schunks.append((w, k, take))
            k += take
    NGC, NSC = len(gchunks), len(schunks)

    def wrap_rep(a):
        return np.tile(a.reshape(-1, 16).T, (8, 1)).astype(np.int16)

    widx = np.zeros((NCORES, NGC, 128, IDXW), np.int16)
    sidx = np.zeros((NCORES, NSC, 128, SIDXW), np.int16)
    for c in range(NCORES):
        for w in range(NW):
            dt, sr, cnt, nrank = per_cw[(c, w)]
            wr = np.empty(NLOCP, np.int64)
            wr[nrank] = np.arange(NLOCP)
            # column offset of each position in the window grid
            C_k = np.concatenate(([0], np.cumsum(D_w[w])[:-1]))
            cols_total = int(np.cumsum(D_w[w])[-1])
            g_arr = np.full(cols_total * 128, PAD_REL, np.int64)
            if dt.size:
                r = wr[dt]
                o = np.lexsort((np.arange(dt.size), r))
                r_s = r[o]
                starts = np.searchsorted(r_s, np.arange(NLOCP))
                j = np.arange(r_s.size) - starts[r_s]
                col = C_k[r_s // 128] + j
                g = col * 128 + (r_s % 128)
                g_arr[g] = sr[o]
            # slice by gather chunks of this window
            for k, (wk, runs, cw) in enumerate(gchunks):
                if wk != w:
                    continue
                parts = []
                for (k0, nk, D) in runs:
                    c0 = int(C_k[k0])
                    parts.append(g_arr[c0 * 128:(c0 + nk * D) * 128])
                ga = np.concatenate(parts) if parts else np.zeros(0, np.int64)
                assert ga.size == cw * 128
                widx[c, k, :, : ga.size // 16] = wrap_rep(ga)
            for k, (wk, p0, npos) in enumerate(schunks):
                if wk != w:
                    continue
                toks = nrank[p0 * 128:(p0 + npos) * 128]
                toks = np.where(toks < NLOC, toks, DUMMY_TOK)
                # stream position g = j*128 + p  <->  wrank (p0+j)*128 + p
                sa = toks.reshape(npos, 128).ravel()
                sidx[c, k, :, : sa.size // 16] = wrap_rep(sa)

    # ---- folded weights (float64 for stability)
    W1 = np.asarray(inputs["W1"], np.float64)
    b1 = np.asarray(inputs["b1"], np.float64)
    W2 = np.asarray(inputs["W2"], np.float64)
    b2 = np.asarray(inputs["b2"], np.float64)
    W3 = np.asarray(inputs["W3"], np.float64)
    b3 = np.asarray(inputs["b3"], np.float64)
    W4 = np.asarray(inputs["W4"], np.float64)
    b4 = np.asarray(inputs["b4"], np.float64)
    lds = [np.asarray(inputs[f"ld{i+1}"], np.float64) for i in range(3)]
    cWs = [np.asarray(inputs[f"cW{i+1}"], np.float64) for i in range(3)]
    cbs = [np.asarray(inputs[f"cb{i+1}"], np.float64) for i in range(3)]

    K01 = np.zeros((H, H))
    KB1 = np.zeros((H, H))
    KB2 = np.zeros((H, H))
    b_emb = b3.copy()
    for i in range(3):
        th = THETAS[i]
        W3Ti = W3[:, i * H:(i + 1) * H].T          # [k, j]
        K01 += (th[1] + th[2]) * W3Ti
        K01 += th[0] * ((lds[i][0][:, None] * cWs[i].T) @ W3Ti)
        alpha = -th[1] * lds[i][1] - th[2] * (lds[i][1] + lds[i][2])
        beta = th[2] * lds[i][1] * lds[i][2]
        KB1 += alpha[:, None] * W3Ti
        KB2 += beta[:, None] * W3Ti
        b_emb += W3[:, i * H:(i + 1) * H] @ cbs[i]
    w4b = np.concatenate([W4.T, b4[None, :]], axis=0)        # [H+1, C]

    meta = dict(N=N, E=E, IN=IN, NLOC=NLOC, NCOLS=NCOLS, NLOCP=NLOCP,
                SHARD=SHARD, TROWS=TROWS, WBASE=WBASE, NW=NW, NGRP=NGRP,
                NGC=NGC, NSC=NSC, gchunks=gchunks, schunks=schunks,
                NPOS=NPOS)

    # ---- per-core input maps
    in_maps = []
    for c in range(NCORES):
        xin = np.zeros((IN, NLOCP), BF16NP)
        xin[:, :NLOC] = in_feat[perm[c]].T.astype(BF16NP)
        dinv = np.zeros((128, NCOLS, 1), np.float32)
        lidx = np.arange(NLOC)
        dinv[lidx % 128, lidx // 128, 0] = d_inv[perm[c]]
        in_maps.append({
            "xin": xin,
            "widx": widx[c].reshape(NGC * 128, IDXW),
            "sidx": sidx[c].reshape(NSC * 128, SIDXW),
            "dinv": dinv,
            "w1t": W1.T.astype(BF16NP).copy(),
            "b1": b1.astype(np.float32).reshape(H, 1),
            "w2t": W2.T.astype(BF16NP).copy(),
            "b2": b2.astype(np.float32).reshape(H, 1),
            "k01": K01.astype(BF16NP),
            "kb1": KB1.astype(BF16NP),
            "kb2": KB2.astype(BF16NP),
            "bemb": b_emb.astype(np.float32).reshape(H, 1),
            "w4b": w4b.astype(BF16NP),
        })
    return meta, in_maps, perm


# ---------------------------------------------------------------- builder

def _build(meta):
    IN = meta["IN"]
    NLOC, NCOLS, NLOCP = meta["NLOC"], meta["NCOLS"], meta["NLOCP"]
    SHARD, TROWS, WBASE = meta["SHARD"], meta["TROWS"], meta["WBASE"]
    NGRP, NGC, NSC = meta["NGRP"], meta["NGC"], meta["NSC"]
    NW = meta["NW"]
    gchunks, schunks = meta["gchunks"], meta["schunks"]
    HC = H + C_OUT
    Relu = mybir.ActivationFunctionType.Relu

    nc = bacc.Bacc("TRN2", target_bir_lowering=False, debug=False,
                   num_devices=NCORES, num_swdge_queues=2)

    xin = nc.dram_tensor("xin", [IN, NLOCP], BF16, kind="ExternalInput")
    widx = nc.dram_tensor("widx", [NGC * 128, IDXW], I16, kind="ExternalInput")
    sidx = nc.dram_tensor("sidx", [NSC * 128, SIDXW], I16, kind="ExternalInput")
    dinv = nc.dram_tensor("dinv", [128, NCOLS, 1], F32, kind="ExternalInput")
    w1t = nc.dram_tensor("w1t", [IN, H], BF16, kind="ExternalInput")
    b1 = nc.dram_tensor("b1", [H, 1], F32, kind="ExternalInput")
    w2t = nc.dram_tensor("w2t", [H, H], BF16, kind="ExternalInput")
    b2 = nc.dram_tensor("b2", [H, 1], F32, kind="ExternalInput")
    k01 = nc.dram_tensor("k01", [H, H], BF16, kind="ExternalInput")
    kb1 = nc.dram_tensor("kb1", [H, H], BF16, kind="ExternalInput")
    kb2 = nc.dram_tensor("kb2", [H, H], BF16, kind="ExternalInput")
    bemb = nc.dram_tensor("bemb", [H, 1], F32, kind="ExternalInput")
    w4b = nc.dram_tensor("w4b", [H + 1, C_OUT], BF16, kind="ExternalInput")
    out = nc.dram_tensor("out", [NLOCP, HC], F32, kind="ExternalOutput")


    with tile.TileContext(nc) as tc:
        with tc.tile_pool(name="dram", bufs=1, space="DRAM") as dram, \
             tc.tile_pool(name="cst", bufs=1) as cst, \
             tc.tile_pool(name="sb", bufs=1) as sb, \
             tc.tile_pool(name="prtp", bufs=2) as prtp, \
             tc.tile_pool(name="msgp", bufs=4) as msgp, \
             tc.tile_pool(name="idxp", bufs=4) as idxp, \
             tc.tile_pool(name="midp", bufs=2) as midp, \
             tc.tile_pool(name="ps", bufs=4, space="PSUM") as ps:

            table1 = dram.tile([TROWS, H], F32, addr_space="Shared")
            table2 = dram.tile([TROWS, H], F32, addr_space="Shared")
            x1s = dram.tile([SHARD, H], F32)
            x2s = dram.tile([SHARD, H], F32)

            # ---------- constants
            w1t_t = cst.tile([IN, H], BF16)
            nc.sync.dma_start(w1t_t[:], w1t[:])
            w2t_t = cst.tile([H, H], BF16)
            nc.sync.dma_start(w2t_t[:], w2t[:])
            b1_t = cst.tile([H, 1], F32)
            nc.sync.dma_start(b1_t[:], b1[:])
            b2_t = cst.tile([H, 1], F32)
            nc.sync.dma_start(b2_t[:], b2[:])
            k01_t = cst.tile([H, H], BF16)
            nc.sync.dma_start(k01_t[:], k01[:])
            kb1_t = cst.tile([H, H], BF16)
            nc.sync.dma_start(kb1_t[:], kb1[:])
            kb2_t = cst.tile([H, H], BF16)
            nc.sync.dma_start(kb2_t[:], kb2[:])
            bemb_t = cst.tile([H, 1], F32)
            nc.sync.dma_start(bemb_t[:], bemb[:])
            w4b_t = cst.tile([H + 1, C_OUT], BF16)
            nc.sync.dma_start(w4b_t[:], w4b[:])
            dinv_t = cst.tile([128, NCOLS, 1], F32)
            nc.sync.dma_start(dinv_t[:], dinv[:])
            d2_t = cst.tile([128, NCOLS, 1], F32)
            nc.vector.tensor_mul(d2_t[:], dinv_t[:], dinv_t[:])
            i64 = cst.tile([H, H], BF16)
            make_identity(nc, i64[:])
            i128 = cst.tile([128, 128], BF16)
            make_identity(nc, i128[:])

            # ---------- MLP (transposed chain)
            xin_t = sb.tile([IN, NLOCP], BF16, tag="bigA")
            nc.sync.dma_start(xin_t[:], xin[:])
            h2T = sb.tile([H, NLOCP], BF16, tag="h2T")
            for s0 in range(0, NLOCP, 512):
                cw = min(512, NLOCP - s0)
                sl = slice(s0, s0 + cw)
                p1 = ps.tile([H, 512], F32, tag="ps")
                nc.tensor.matmul(p1[:, :cw], lhsT=w1t_t[:], rhs=xin_t[:, sl],
                                 start=True, stop=True)
                ht = midp.tile([H, 512], BF16, tag="ht")
                nc.scalar.activation(ht[:, :cw], p1[:, :cw], Relu, bias=b1_t[:])
                p2 = ps.tile([H, 512], F32, tag="ps")
                nc.tensor.matmul(p2[:, :cw], lhsT=w2t_t[:], rhs=ht[:, :cw],
                                 start=True, stop=True)
                nc.scalar.activation(h2T[:, sl], p2[:, :cw], Relu, bias=b2_t[:])

            # ---------- x1 = h * d (node-major), staged then DMA'd out
            x1_sb = sb.tile([128, NCOLS, H], F32, tag="xsb")
            for g0 in range(0, NCOLS, 7):
                gn = min(7, NCOLS - g0)
                px = ps.tile([128, 7 * H], F32, tag="ps")
                for t in range(gn):
                    col = g0 + t
                    nc.tensor.matmul(px[:, t * H:(t + 1) * H],
                                     lhsT=h2T[:, col * 128:(col + 1) * 128],
                                     rhs=i64[:], start=True, stop=True)
                nc.vector.tensor_mul(
                    x1_sb[:, g0:g0 + gn, :],
                    px[:, :gn * H].rearrange("p (c h) -> p c h", h=H),
                    dinv_t[:, g0:g0 + gn, :].to_broadcast([128, gn, H]))
            nc.sync.dma_start(
                x1s[:].rearrange("(c p) h -> p c h", p=128), x1_sb[:])
            nc.gpsimd.collective_compute(
                "AllGather", mybir.AluOpType.bypass,
                replica_groups=[list(range(NCORES))],
                ins=[x1s[:].opt()], outs=[table1[:].opt()])

            # ---------- aggregation steps
            qctr = [0]

            def next_q():
                qctr[0] += 1
                return qctr[0] % 2

            def agg_step(own_tag, peer_tag, table):
                own = sb.tile([128, NGRP, H], F32, tag=own_tag, name=own_tag)
                peer = sb.tile([128, NGRP, H], F32, tag=peer_tag, name=peer_tag)
                nc.vector.memset(own[:], 0.0)
                nc.vector.memset(peer[:], 0.0)
                parts = {}

                def do_gather(k, w, runs, cw):
                    size = cw * 128
                    gi = idxp.tile([128, IDXW], I16, tag="gi", name="gi")
                    nc.sync.dma_start(gi[:], widx[k * 128:(k + 1) * 128, :])
                    msg = msgp.tile([128, GCH_COLS, H], F32, tag="msg",
                                    name="msg")
                    nc.gpsimd.dma_gather(
                        out_ap=msg[:, :cw, :],
                        in_ap=table[w * WBASE:, :],
                        idxs_ap=gi[:, : size // 16],
                        num_idxs=size, num_idxs_reg=size,
                        elem_size=H, queue_num=next_q(),
                        single_packet=False)
                    c_off = 0
                    for (k0, nk, D) in runs:
                        kk0, nnk = k0, nk
                        while nnk > 0:
                            sci = kk0 // SCH_POS
                            pp = parts.get((w, sci))
                            if pp is None:
                                pp = prtp.tile([128, SCH_POS, H], F32,
                                               tag="part", name="part")
                                parts[(w, sci)] = pp
                            take = min(nnk, (sci + 1) * SCH_POS - kk0)
                            src = msg[:, c_off:c_off + take * D, :].rearrange(
                                "p (k d) h -> p k h d", d=D)
                            nc.vector.tensor_reduce(
                                pp[:, kk0 - sci * SCH_POS:
                                   kk0 - sci * SCH_POS + take, :],
                                src, axis=mybir.AxisListType.X,
                                op=mybir.AluOpType.add)
                            c_off += take * D
                            kk0 += take
                            nnk -= take

                def do_scatter(k, w, p0, npos):
                    size = npos * 128
                    si = idxp.tile([128, SIDXW], I16, tag="si", name="si")
                    nc.sync.dma_start(si[:], sidx[k * 128:(k + 1) * 128, :])
                    part = parts[(w, p0 // SCH_POS)]
                    nc.gpsimd.dma_scatter_add(
                        out_ap=own[:],
                        in_ap=part[:, :npos, :],
                        idxs_ap=si[:, : size // 16],
                        num_idxs=size, num_idxs_reg=size,
                        elem_size=H,
                        sbuf_tokens_per_rank=128,
                        parity_reg=0,
                        out_ap_other=peer[:],
                        queue_num=next_q(), single_packet=False)

                for w in range(NW):
                    wg = [(k, runs, cw) for k, (wk, runs, cw)
                          in enumerate(gchunks) if wk == w]
                    ws = [(k, p0, npos) for k, (wk, p0, npos)
                          in enumerate(schunks) if wk == w]
                    # issue each scatter once the gather chunks covering its
                    # positions have been issued (reduces trail them closely)
                    pos_done = 0
                    si = 0
                    for (k, runs, cw) in wg:
                        do_gather(k, w, runs, cw)
                        pos_done = max(pos_done,
                                       max(k0 + nk for k0, nk, _ in runs))
                        while si < len(ws) and \
                                ws[si][1] + ws[si][2] <= pos_done - 32:
                            do_scatter(ws[si][0], w, ws[si][1], ws[si][2])
                            si += 1
                    for (k, p0, npos) in ws[si:]:
                        do_scatter(k, w, p0, npos)
                return own, peer

            def par_view(ap3, par):
                # [128, NCOLS, X] -> the even/odd columns [128, NGRP, X]
                v = ap3.rearrange("p (c t) x -> p c t x", t=2)
                return v[:, :, par:par + 1, :].rearrange("p c o x -> p c (o x)")

            own1, peer1 = agg_step("accA", "accB", table1)

            # x2 = M1 * d^2  (M1 even cols in own1, odd cols in peer1)
            x2_sb = sb.tile([128, NCOLS, H], F32, tag="xsb")
            for par, acc in ((0, own1), (1, peer1)):
                nc.vector.tensor_mul(
                    par_view(x2_sb[:], par), acc[:],
                    par_view(d2_t[:], par).to_broadcast([128, NGRP, H]))
            nc.sync.dma_start(
                x2s[:].rearrange("(c p) h -> p c h", p=128), x2_sb[:])
            nc.gpsimd.collective_compute(
                "AllGather", mybir.AluOpType.bypass,
                replica_groups=[list(range(NCORES))],
                ins=[x2s[:].opt()], outs=[table2[:].opt()])

            # B1 = M1 * d (node-major bf16; transposed on demand at mm4)
            b1a = sb.tile([128, NGRP, H], BF16, tag="b1a", name="b1a")
            b1b = sb.tile([128, NGRP, H], BF16, tag="b1b", name="b1b")
            for par, acc, dst in ((0, own1, b1a), (1, peer1, b1b)):
                nc.vector.tensor_mul(
                    dst[:], acc[:],
                    par_view(dinv_t[:], par).to_broadcast([128, NGRP, H]))

            own2, peer2 = agg_step("accA", "accB", table2)
            b2a = sb.tile([128, NGRP, H], BF16, tag="b2a", name="b2a")
            b2b = sb.tile([128, NGRP, H], BF16, tag="b2b", name="b2b")
            for par, acc, dst in ((0, own2, b2a), (1, peer2, b2b)):
                nc.vector.tensor_mul(
                    dst[:], acc[:],
                    par_view(dinv_t[:], par).to_broadcast([128, NGRP, H]))

            # ---------- emb (transposed) with on-demand B transposes
            embT = sb.tile([H + 1, NLOCP], BF16, tag="xsb")
            nc.vector.memset(embT[H:H + 1, :], 1.0)
            for s0 in range(0, NLOCP, 512):
                cw = min(512, NLOCP - s0)
                ncols_here = cw // 128
                b1T = midp.tile([H, 512], BF16, tag="b1T", name="b1T")
                b2T = midp.tile([H, 512], BF16, tag="b2T", name="b2T")
                pt1 = ps.tile([H, 512], BF16, tag="pt", bufs=4)
                pt2 = ps.tile([H, 512], BF16, tag="pt", bufs=4)
                for t in range(ncols_here):
                    col = s0 // 128 + t
                    srcs = (b1a if col % 2 == 0 else b1b,
                            b2a if col % 2 == 0 else b2b)
                    g2 = col // 2
                    nc.tensor.transpose(
                        pt1[:, t * 128:(t + 1) * 128],
                        srcs[0][:, g2:g2 + 1, :].rearrange("p o h -> p (o h)"),
                        i128[:])
                    nc.tensor.transpose(
                        pt2[:, t * 128:(t + 1) * 128],
                        srcs[1][:, g2:g2 + 1, :].rearrange("p o h -> p (o h)"),
                        i128[:])
                nc.vector.tensor_copy(b1T[:, :cw], pt1[:, :cw])
                nc.vector.tensor_copy(b2T[:, :cw], pt2[:, :cw])
                pe = ps.tile([H, 512], F32, tag="ps")
                sl = slice(s0, s0 + cw)
                nc.tensor.matmul(pe[:, :cw], lhsT=k01_t[:], rhs=h2T[:, sl],
                                 start=True, stop=False)
                nc.tensor.matmul(pe[:, :cw], lhsT=kb1_t[:], rhs=b1T[:, :cw],
                                 start=False, stop=False)
                nc.tensor.matmul(pe[:, :cw], lhsT=kb2_t[:], rhs=b2T[:, :cw],
                                 start=False, stop=True)
                nc.scalar.activation(embT[0:H, sl], pe[:, :cw], Relu,
                                     bias=bemb_t[:])

            # ---------- outputs: emb node-major + logits, packed [NLOC, 66]
            outb = sb.tile([128, NCOLS, HC], F32, tag="bigA")
            for g0 in range(0, NCOLS, 7):
                gn = min(7, NCOLS - g0)
                po = ps.tile([128, 7 * HC], F32, tag="ps")
                for t in range(gn):
                    col = g0 + t
                    tsl = slice(col * 128, (col + 1) * 128)
                    off = t * HC
                    nc.tensor.matmul(po[:, off:off + H],
                                     lhsT=embT[0:H, tsl], rhs=i64[:],
                                     start=True, stop=True)
                    nc.tensor.matmul(po[:, off + H:off + HC],
                                     lhsT=embT[:, tsl], rhs=w4b_t[:],
                                     start=True, stop=True)
                nc.vector.tensor_copy(
                    outb[:, g0:g0 + gn, :],
                    po[:, :gn * HC].rearrange("p (c h) -> p c h", h=HC))
            nc.sync.dma_start(
                out[:].rearrange("(c p) h -> p c h", p=128), outb[:])

    nc.compile()
    return nc


# ---------------------------------------------------------------- entry

def run(inputs, trace=False, trace_kwargs=None):
    meta, in_maps, perm = _prep(inputs)
    nc = _build(meta)
    kw = {}
    if trace:
        kw["trace"] = True
        if trace_kwargs:
            kw["trace_kwargs"] = trace_kwargs
    res = run_bass_kernel_spmd(nc, in_maps, core_ids=list(range(NCORES)), **kw)
    N = meta["N"]
    logits = np.zeros((N, C_OUT), np.float32)
    emb = np.zeros((N, H), np.float32)
    NLOC = meta["NLOC"]
    for c in range(NCORES):
        o = np.asarray(res.results[c]["out"])[:NLOC]
        emb[perm[c]] = o[:, :H]
        logits[perm[c]] = o[:, H:]
    return (logits, emb), res


def kernel(**inputs):
    (logits, emb), _ = run(inputs)
    return logits, emb
